# revision 17
# baseline (speedup 1.0000x reference)
# Trainium2 Bass kernel for the non-local attention block (nn_DRAL_88476326297980).
#
# Reference computation (per batch b):
#   theta = theta_w @ x_b + theta_b            (CI=128, N=4096)
#   phi   = maxpool2x2(phi_w @ y_b + phi_b)    (CI=128, P=1024)
#   g     = maxpool2x2(g_w  @ y_b + g_b)       (CI=128, P=1024)
#   f     = theta^T @ phi                      (N, P)
#   fdiv  = softmax(f, axis=P)
#   z     = fdiv @ g^T                         (N, CI)
#   wz    = W_w @ z^T + W_b                    (C=256, N)
#   out   = BN(wz over all b,n) + x            (training-mode batch stats)
#
# Sharding: data-parallel over batch, 2 batches per core, 8 cores.
# BN batch statistics are combined with a tiny (128x4) AllReduce.
#
# Math simplifications used (exact, not approximations):
#  - phi_b adds a per-row constant to f -> softmax-invariant -> dropped.
#  - g_b adds a per-CI constant to z (softmax weights sum to 1) -> shifts wz
#    per-channel -> cancelled by the BN mean subtraction -> dropped.
#  - W_b shifts wz per-channel -> cancelled by BN mean subtraction -> dropped.
#  - BN statistics are computed from z (before the W conv):
#      sum(wz)_c  = W @ sum_m(z_m)     (sum_m z from accum on the zt copies)
#      sum(wz^2)_c = diag(W ZZ W^T)    (ZZ = z Gram matrix, accumulated in PSUM)
#    so the AllReduce fires right after the last attention tile and the
#    W conv + normalize + residual + store run as one fused pipeline.
#
# Layout choices:
#  - everything on the PE runs in bf16 (full rate at any tile size; f32r pays
#    4x below 256 output cols, which hit the old Gram/transpose path).
#  - f is computed TRANSPOSED (fT: pooled dim on partitions, n on free) so both
#    attention matmuls contract over the partition dim with no transposes of f.
#  - softmax denominators come from an extra all-ones column appended to the
#    g^T tiles: the z-matmul then yields [z_unnorm | s | junk] in one PSUM
#    accumulation group.
#  - x and y are loaded as bf16 (host-converted), halving input HBM traffic.

import numpy as np
from ml_dtypes import bfloat16 as ml_bf16

import concourse.bass as bass
import concourse.mybir as mybir
import concourse.tile as tile
from concourse import bacc
from concourse.bass_utils import run_bass_kernel_spmd

F32 = mybir.dt.float32
F32R = mybir.dt.float32r
BF16 = mybir.dt.bfloat16
ALU = mybir.AluOpType
ACT = mybir.ActivationFunctionType
AX = mybir.AxisListType

NCORES = 8
B = 16
BLOC = B // NCORES          # 2 batches per core
C = 256                     # in channels
CI = 128                    # inter channels
N = 4096                    # h*w
MC = 512                    # m-chunk (columns per matmul)
NMC = N // MC               # 8
GW = 132                    # z-matmul output width: 128 z cols + s + 3 junk
EPS = 1e-5
COUNT = B * N               # BN sample count per channel


def build_body(tc, io):
    nc = tc.nc
    x, y, wpack, vpack, gpad, out = (
        io["x"], io["y"], io["wpack"], io["vpack"], io["gpad"], io["out"],
    )

    ctx = io["ctx"]
    consts = ctx.enter_context(tc.tile_pool(name="consts", bufs=1))
    xfp = ctx.enter_context(tc.tile_pool(name="xfp", bufs=2))
    yin = ctx.enter_context(tc.tile_pool(name="yin", bufs=8))
    thp = ctx.enter_context(tc.tile_pool(name="thp", bufs=2))
    poolp = ctx.enter_context(tc.tile_pool(name="poolp", bufs=2))
    ptmp = ctx.enter_context(tc.tile_pool(name="ptmp", bufs=1))
    gtp = ctx.enter_context(tc.tile_pool(name="gtp", bufs=2))
    fxp = ctx.enter_context(tc.tile_pool(name="fxp", bufs=2))
    znp = ctx.enter_context(tc.tile_pool(name="znp", bufs=6))
    ztp = ctx.enter_context(tc.tile_pool(name="ztp", bufs=2))
    wzp = ctx.enter_context(tc.tile_pool(name="wzp", bufs=16))
    wnp = ctx.enter_context(tc.tile_pool(name="wnp", bufs=4))
    outp = ctx.enter_context(tc.tile_pool(name="outp", bufs=6))
    psf = ctx.enter_context(tc.tile_pool(name="psf", bufs=2, space="PSUM"))
    pbank = ctx.enter_context(tc.tile_pool(name="pbank", bufs=3, space="PSUM"))
    pzz = ctx.enter_context(tc.tile_pool(name="pzz", bufs=1, space="PSUM"))
    dram = ctx.enter_context(tc.tile_pool(name="dram", bufs=1, space="DRAM"))

    # ---- constants / weights: two packed DMAs to keep sync fan-in tiny ----
    # wpack (128, 1152) bf16:
    #   [twT(2x128) pwT(2x128) gwT(2x128) wwT(256) identb(128)]
    wp_s = consts.tile([128, 1152], BF16)
    nc.sync.dma_start(out=wp_s, in_=wpack)
    tw_s = wp_s[:, 0:256].rearrange("p (k c) -> p k c", k=2)
    pw_s = wp_s[:, 256:512].rearrange("p (k c) -> p k c", k=2)
    gw_s = wp_s[:, 512:768].rearrange("p (k c) -> p k c", k=2)
    ww_s = wp_s[:, 768:1024]
    identb_s = wp_s[:, 1024:1152]
    # vpack (128, 261) f32: [tb, gamma(2), beta(2), wraw(2x128), wwT(256)]
    vp_s = consts.tile([128, 261], F32)
    nc.sync.dma_start(out=vp_s, in_=vpack)
    tb_s = vp_s[:, 0:1]
    gamma_s = vp_s[:, 1:3]
    beta_s = vp_s[:, 3:5]
    wraw_s = vp_s[:, 5:261].rearrange("p (k c) -> p k c", k=2)
    wwr_s = consts.tile([128, 256], F32R)
    nc.sync.dma_start(out=wwr_s, in_=io["wwr"])

    acc_s = consts.tile([128, BLOC * NMC], F32)       # per (b, mc) z sums
    zz_ps = pzz.tile([128, 128], F32)                 # z Gram matrix accumulator

    zz_n = [0]
    zz_last = BLOC * NMC * 4 - 1                      # 64 accumulated Gram matmuls

    # ---------------- input DMAs for both items, issued upfront -------------
    # y[b] first (the attention loop needs the full phi before it can start),
    # then x[b]; item 1's loads stream in behind item 0's while item 0
    # computes, so the item transition pays no DMA latency.
    x_tiles = {}
    y_tiles_all = {}
    for b in range(BLOC):
        ys = y[b].rearrange("(k p) m -> p k m", p=128)
        y_tiles_all[b] = []
        for q in range(4):
            yr = yin.tile([128, 2, 2 * MC], BF16, tag="yin", name=f"y_{b}_{q}")
            nc.sync.dma_start(out=yr, in_=ys[:, :, q * 2 * MC:(q + 1) * 2 * MC])
            y_tiles_all[b].append(yr)

        x_t = xfp.tile([128, 2, N], BF16, tag="xf", name=f"x_{b}")
        x_tiles[b] = x_t
        xs = x[b].rearrange("(k p) m -> p k m", p=128)
        for q in range(4):
            qs = slice(q * (N // 4), (q + 1) * (N // 4))
            nc.sync.dma_start(out=x_t[:, :, qs], in_=xs[:, :, qs])

    zt_tiles = {}
    phi_tiles = {}
    gt_tiles = {}
    theta_tiles = {}

    def conv_phase(b):
        x_t = x_tiles[b]
        y_tiles = y_tiles_all[b]

        # ---------------- phi/g convs + 2x2 maxpool ----------------
        # pooled tensors: (128ci, 32ph, 32pw)
        phi_p = poolp.tile([128, 32, 32], F32R, tag="phi_p")
        g_p = poolp.tile([128, 32, 32], BF16, tag="g_p")
        for mc in range(NMC):
            yr = y_tiles[mc // 2]
            half = slice((mc % 2) * MC, (mc % 2 + 1) * MC)
            for which, w_s, dst in (("phi", pw_s, phi_p), ("g", gw_s, g_p)):
                cps = pbank.tile([128, MC], F32, tag="bank", name=f"cps_{which}")
                nc.tensor.matmul(cps, w_s[:, 0, :], yr[:, 0, half], start=True, stop=False)
                nc.tensor.matmul(cps, w_s[:, 1, :], yr[:, 1, half], start=False, stop=True)
                # 2x2 maxpool in one reduce: (128, 4ph, 32pw, 2hh, 2ww) -> XY
                v = cps.rearrange("p (ph hh pw ww) -> p ph pw hh ww", ph=4, hh=2, ww=2)
                nc.vector.tensor_reduce(
                    out=dst[:, mc * 4:(mc + 1) * 4, :], in_=v, axis=AX.XY, op=ALU.max,
                )

        # ---------------- theta conv: (128ci, 4096) ----------------
        theta = thp.tile([128, N], F32R, tag="theta")
        for mc in range(NMC):
            ms = slice(mc * MC, (mc + 1) * MC)
            tps = pbank.tile([128, MC], F32, tag="bank", name="tps")
            nc.tensor.matmul(tps, tw_s[:, 0, :], x_t[:, 0, ms], start=True, stop=False)
            nc.tensor.matmul(tps, tw_s[:, 1, :], x_t[:, 1, ms], start=False, stop=True)
            nc.scalar.activation(theta[:, ms], tps, ACT.Identity, bias=tb_s, scale=1.0)

        # ---------------- gT tiles with [ones | zeros] pad columns ----------
        # gt: (128 pooled, 8 pchunk, 132) ; [:, :, 0:128]=g^T, col 128=1, rest 0
        gt = gtp.tile([128, 8, GW], BF16, tag="gt")
        nc.sync.dma_start(out=gt[:, :, 128:GW], in_=gpad)
        g_flat = g_p.rearrange("p a b -> p (a b)")
        for half in range(2):
            gtps = pbank.tile([128, 4, 128], BF16, tag="bank", name="gtps")
            for j in range(4):
                pch = half * 4 + j
                nc.tensor.transpose(
                    gtps[:, j, :], g_flat[:, pch * 128:(pch + 1) * 128],
                    identb_s,
                )
            nc.vector.tensor_copy(out=gt[:, half * 4:(half + 1) * 4, 0:128], in_=gtps)

        phi_tiles[b] = phi_p
        gt_tiles[b] = gt
        theta_tiles[b] = theta

    # ---------------- attention per item / m-chunk ----------------
    # item b+1's conv phase is injected midway through item b's attention so
    # neither the PE nor the ACT exp chain drains at the item boundary, while
    # attention 0 still starts as soon as item 0's inputs land.
    def attention_phase(b, inject_mc=None, inject_fn=None):
        theta = theta_tiles[b]
        zt = ztp.tile([128, N], BF16, tag="zt", name=f"zt_{b}")
        zt_tiles[b] = zt
        phi_flat = phi_tiles[b].rearrange("p a b -> p (a b)")
        gt = gt_tiles[b]
        for mc in range(NMC):
            if mc == inject_mc and inject_fn is not None:
                inject_fn()
            ms = slice(mc * MC, (mc + 1) * MC)
            # fT tiles: (128 pooled, 512 m) for each of 8 pooled chunks; exp on ACT
            fexp = fxp.tile([128, 8, MC], BF16, tag="fexp")
            for half in range(4):
                fps = psf.tile([128, 2, MC], F32, tag="f")
                for i in range(2):
                    pch = half * 2 + i
                    nc.tensor.matmul(
                        fps[:, i, :],
                        phi_flat[:, pch * 128:(pch + 1) * 128],
                        theta[:, ms],
                        start=True, stop=True,
                    )
                nc.scalar.activation(fexp[:, 2 * half:2 * half + 2, :], fps, ACT.Exp)

            # z matmuls: out (128 m, [z | s | junk]) accumulated over 8 pooled
            # chunks; two m-subtiles per PSUM bank tile
            tp = pbank.tile([128, 4, 128], BF16, tag="bank", name="tp")
            for j2 in range(2):
                zb = pbank.tile([128, 512], F32, tag="bank", name="zb")
                for i in range(2):
                    sub = j2 * 2 + i
                    for pch in range(8):
                        nc.tensor.matmul(
                            zb[:, i * 256:i * 256 + GW],
                            fexp[:, pch, sub * 128:(sub + 1) * 128],
                            gt[:, pch, :],
                            start=(pch == 0), stop=(pch == 7),
                        )
                zb2 = zb.rearrange("p (i c) -> p i c", i=2)
                rc = ptmp.tile([128, 2], F32, tag="rc", bufs=4)
                nc.vector.reciprocal(rc, zb2[:, :, 128])
                zn2 = znp.tile([128, 2, 128], BF16, tag="zn")
                nc.vector.tensor_tensor(
                    zn2, zb2[:, :, 0:128],
                    rc[:, :, None].to_broadcast((128, 2, 128)), ALU.mult,
                )
                for i in range(2):
                    sub = j2 * 2 + i
                    nc.tensor.transpose(tp[:, sub, :], zn2[:, i, :], identb_s)
                    # Gram accumulation for BN variance: ZZ += z_m^T z_m
                    nc.tensor.matmul(
                        zz_ps, zn2[:, i, :], zn2[:, i, :],
                        start=(zz_n[0] == 0), stop=(zz_n[0] == zz_last),
                        skip_group_check=True,
                    )
                    zz_n[0] += 1
            # zt copy doubles as the per-chunk z column-sum for the BN mean
            nc.vector.tensor_scalar(
                zt[:, ms], tp.rearrange("p a b -> p (a b)"), 1.0, 0.0,
                ALU.mult, ALU.add, accum_out=acc_s[:, b * NMC + mc:b * NMC + mc + 1],
            )


    conv_phase(0)
    attention_phase(0, inject_mc=4, inject_fn=lambda: conv_phase(1))
    attention_phase(1)

    # ---------------- global BN stats via AllReduce ----------------
    # ls[:, cc] = sum(wz), ls[:, 2+cc] = sum(wz^2), both derived from z
    ls = consts.tile([128, 4], F32)
    sumz = consts.tile([128, 2], F32R)
    with nc.allow_low_precision(reason="f32r is full-width f32 storage"):
        nc.vector.reduce_sum(out=sumz[:, 0:1], in_=acc_s, axis=AX.X)
        nc.vector.reduce_sum(out=sumz[:, 1:2], in_=acc_s[:, 0:1], axis=AX.X)
    # sum(wz)_c = W[c,:] @ sumz ; sum(wz^2)_c = diag(W ZZ W^T)
    zz_s = consts.tile([128, 128], F32R)
    nc.vector.tensor_copy(out=zz_s, in_=zz_ps)
    for cc in range(2):
        s1_ps = pbank.tile([128, 2], F32, tag="bank", name="s1_ps")
        nc.tensor.matmul(s1_ps, wwr_s[:, cc * 128:(cc + 1) * 128], sumz,
                         start=True, stop=True)
        nc.vector.tensor_copy(out=ls[:, cc:cc + 1], in_=s1_ps[:, 0:1])
        u_ps = pbank.tile([128, 128], F32, tag="bank", name="u_ps")
        nc.tensor.matmul(u_ps, wwr_s[:, cc * 128:(cc + 1) * 128], zz_s,
                         start=True, stop=True)
        qjunk = ptmp.tile([128, 128], F32, tag="qjunk", bufs=1)
        nc.vector.scalar_tensor_tensor(
            qjunk, u_ps, 1.0, wraw_s[:, cc, :], ALU.mult, ALU.mult,
            accum_out=ls[:, 2 + cc:3 + cc],
        )

    cc_in = dram.tile([128, 4], F32)
    cc_out = dram.tile([128, 4], F32)
    nc.sync.dma_start(out=cc_in, in_=ls)
    if io.get("single_core_sim"):
        # stand-in for the AllReduce so TimelineSim (single-core) can run
        nc.sync.dma_start(out=cc_out, in_=cc_in)
    else:
        nc.gpsimd.collective_compute(
            "AllReduce", ALU.add,
            replica_groups=[list(range(NCORES))],
            ins=[cc_in.opt()], outs=[cc_out.opt()],
        )
    gs = consts.tile([128, 4], F32)
    nc.sync.dma_start(out=gs, in_=cc_out)

    inv = 1.0 / COUNT
    mean = consts.tile([128, 2], F32)
    nc.vector.tensor_scalar(mean, gs[:, 0:2], inv, None, ALU.mult)
    e2 = consts.tile([128, 2], F32)
    nc.vector.tensor_scalar(e2, gs[:, 2:4], inv, None, ALU.mult)
    msq = consts.tile([128, 2], F32)
    nc.vector.tensor_mul(msq, mean, mean)
    u = consts.tile([128, 2], F32)
    nc.vector.tensor_sub(u, e2, msq)
    nc.vector.tensor_scalar(u, u, EPS, None, ALU.add)
    # rsqrt(u) = exp(-0.5*ln(u)) -- Ln/Exp share the softmax's ACT table set
    y0 = consts.tile([128, 2], F32)
    nc.scalar.activation(y0, u, ACT.Ln)
    r0 = consts.tile([128, 2], F32)
    nc.scalar.activation(r0, y0, ACT.Exp, scale=-0.5)
    a_s = consts.tile([128, 2], F32)
    nc.vector.tensor_mul(a_s, r0, gamma_s)
    nb = consts.tile([128, 2], F32)
    nc.vector.tensor_mul(nb, mean, a_s)
    nc.vector.tensor_sub(nb, beta_s, nb)

    # ---------------- normalize + residual + store ----------------
    for b in range(BLOC):
        x_t = x_tiles[b]
        zt = zt_tiles[b]
        for mc in range(NMC):
            ms = slice(mc * MC, (mc + 1) * MC)
            for cc in range(2):
                csl = slice(cc * 128, (cc + 1) * 128)
                wb = pbank.tile([128, MC], F32, tag="bank", name="wb")
                nc.tensor.matmul(
                    wb, ww_s[:, cc * 128:(cc + 1) * 128], zt[:, ms],
                    start=True, stop=True,
                )
                # normalize on ACT: wn = wz*a + (beta - mean*a)
                wn = wnp.tile([128, MC], BF16, tag="wn")
                nc.scalar.activation(
                    wn, wb, ACT.Identity,
                    bias=nb[:, cc:cc + 1], scale=a_s[:, cc:cc + 1],
                )
                ot = outp.tile([128, MC], F32, tag="ot")
                eng = nc.vector if (mc + cc) % 2 == 0 else nc.gpsimd
                eng.tensor_add(ot, wn, x_t[:, cc, ms])
                nc.sync.dma_start(out=out[b, csl, ms], in_=ot)


def make_io(nc):
    return {
        "x": nc.dram_tensor("x", [BLOC, C, N], BF16, kind="ExternalInput").ap(),
        "y": nc.dram_tensor("y", [BLOC, C, N], BF16, kind="ExternalInput").ap(),
        "wpack": nc.dram_tensor("wpack", [128, 1152], BF16, kind="ExternalInput").ap(),
        "vpack": nc.dram_tensor("vpack", [128, 261], F32, kind="ExternalInput").ap(),
        "wwr": nc.dram_tensor("wwr", [128, 256], mybir.dt.float32r, kind="ExternalInput").ap(),
        "gpad": nc.dram_tensor("gpad", [128, 8, 4], BF16, kind="ExternalInput").ap(),
        "out": nc.dram_tensor("out", [BLOC, C, N], F32, kind="ExternalOutput").ap(),
    }


_CACHE = {}


def _get_program():
    if "nc" in _CACHE:
        return _CACHE["nc"], _CACHE["io"]
    nc = bacc.Bacc(
        "TRN2", target_bir_lowering=False, debug=False,
        enable_asserts=False, num_devices=NCORES,
    )
    io = make_io(nc)
    from contextlib import ExitStack
    with tile.TileContext(nc) as tc:
        with ExitStack() as ctx:
            io["ctx"] = ctx
            build_body(tc, io)
    nc.compile()
    _CACHE["nc"] = nc
    _CACHE["io"] = io
    return nc, io


def kernel(x, y, theta_w, theta_b, phi_w, phi_b, g_w, g_b, W_w, W_b,
           bn_gamma, bn_beta, _trace=False, **_unused):
    x = np.asarray(x, dtype=np.float32).reshape(B, C, N).astype(ml_bf16)
    y = np.asarray(y, dtype=np.float32).reshape(B, C, N).astype(ml_bf16)

    def chunked(wT):
        # (C, CI) -> (128, 2, CI): [p, k, ci] = wT[k*128+p, ci]
        return np.asarray(wT, np.float32).reshape(2, 128, CI).transpose(1, 0, 2)

    tw = chunked(np.asarray(theta_w, np.float32).T)
    pw = chunked(np.asarray(phi_w, np.float32).T)
    gw = chunked(np.asarray(g_w, np.float32).T)
    ww = np.asarray(W_w, np.float32).T                             # (CI, C)
    wraw = chunked(np.asarray(W_w, np.float32))                    # c-part layout
    ident = np.eye(128, dtype=np.float32)
    wpack = np.ascontiguousarray(np.concatenate([
        tw.reshape(128, 256), pw.reshape(128, 256), gw.reshape(128, 256),
        ww, ident], axis=1).astype(ml_bf16))
    tb = np.asarray(theta_b, np.float32).reshape(CI, 1)
    gamma = np.asarray(bn_gamma, np.float32).reshape(2, 128).T
    beta = np.asarray(bn_beta, np.float32).reshape(2, 128).T
    vpack = np.ascontiguousarray(np.concatenate(
        [tb, gamma, beta, wraw.reshape(128, 256)], axis=1))
    wwr = np.ascontiguousarray(ww)
    gpad = np.zeros((128, 8, 4), ml_bf16)
    gpad[:, :, 0] = 1.0
    # phi_b, g_b, W_b intentionally unused: softmax-invariant / cancelled by BN.

    nc, _ = _get_program()
    in_maps = []
    for k in range(NCORES):
        in_maps.append({
            "x": np.ascontiguousarray(x[k * BLOC:(k + 1) * BLOC]),
            "y": np.ascontiguousarray(y[k * BLOC:(k + 1) * BLOC]),
            "wpack": wpack, "vpack": vpack, "wwr": wwr, "gpad": gpad,
        })
    res = run_bass_kernel_spmd(nc, in_maps, core_ids=list(range(NCORES)), trace=_trace)
    out = np.concatenate([r_["out"] for r_ in res.results], axis=0)
    if _trace:
        _CACHE["last_results"] = res
    return out.reshape(B, C, 64, 64)


# revision 30
# speedup vs baseline: 1.0332x; 1.0332x over previous
# Trainium2 Bass kernel for the non-local attention block (nn_DRAL_88476326297980).
#
# Reference computation (per batch b):
#   theta = theta_w @ x_b + theta_b            (CI=128, N=4096)
#   phi   = maxpool2x2(phi_w @ y_b + phi_b)    (CI=128, P=1024)
#   g     = maxpool2x2(g_w  @ y_b + g_b)       (CI=128, P=1024)
#   f     = theta^T @ phi                      (N, P)
#   fdiv  = softmax(f, axis=P)
#   z     = fdiv @ g^T                         (N, CI)
#   wz    = W_w @ z^T + W_b                    (C=256, N)
#   out   = BN(wz over all b,n) + x            (training-mode batch stats)
#
# Sharding: data-parallel over batch, 2 batches per core, 8 cores.
# BN batch statistics are combined with a tiny (128x4) AllReduce.
#
# Math simplifications used (exact, not approximations):
#  - phi_b adds a per-row constant to f -> softmax-invariant -> dropped.
#  - g_b adds a per-CI constant to z (softmax weights sum to 1) -> shifts wz
#    per-channel -> cancelled by the BN mean subtraction -> dropped.
#  - W_b shifts wz per-channel -> cancelled by BN mean subtraction -> dropped.
#  - BN statistics are computed from z (before the W conv):
#      sum(wz)_c  = W @ sum_m(z_m)     (sum_m z from accum on the zt copies)
#      sum(wz^2)_c = diag(W ZZ W^T)    (ZZ = z Gram matrix, accumulated in PSUM)
#    so the AllReduce fires right after the last attention tile and the
#    W conv + normalize + residual + store run as one fused pipeline.
#
# Layout choices:
#  - everything on the PE runs in bf16 (full rate at any tile size; f32r pays
#    4x below 256 output cols, which hit the old Gram/transpose path).
#  - f is computed TRANSPOSED (fT: pooled dim on partitions, n on free) so both
#    attention matmuls contract over the partition dim with no transposes of f.
#  - softmax denominators come from an extra all-ones column appended to the
#    g^T tiles: the z-matmul then yields [z_unnorm | s | junk] in one PSUM
#    accumulation group.
#  - x and y are loaded as bf16 (host-converted), halving input HBM traffic.

import numpy as np
from ml_dtypes import bfloat16 as ml_bf16

import concourse.bass as bass
import concourse.mybir as mybir
import concourse.tile as tile
from concourse import bacc
from concourse.bass_utils import run_bass_kernel_spmd

F32 = mybir.dt.float32
F32R = mybir.dt.float32r
BF16 = mybir.dt.bfloat16
ALU = mybir.AluOpType
ACT = mybir.ActivationFunctionType
AX = mybir.AxisListType

NCORES = 8
B = 16
BLOC = B // NCORES          # 2 batches per core
C = 256                     # in channels
CI = 128                    # inter channels
N = 4096                    # h*w
MC = 512                    # m-chunk (columns per matmul)
NMC = N // MC               # 8
GW = 132                    # z-matmul output width: 128 z cols + s + 3 junk
EPS = 1e-5
COUNT = B * N               # BN sample count per channel


def build_body(tc, io):
    nc = tc.nc
    x, y, wpack, vpack, gpad, out = (
        io["x"], io["y"], io["wpack"], io["vpack"], io["gpad"], io["out"],
    )

    ctx = io["ctx"]
    consts = ctx.enter_context(tc.tile_pool(name="consts", bufs=1))
    xfp = ctx.enter_context(tc.tile_pool(name="xfp", bufs=2))
    yin = ctx.enter_context(tc.tile_pool(name="yin", bufs=8))
    thp = ctx.enter_context(tc.tile_pool(name="thp", bufs=2))
    poolp = ctx.enter_context(tc.tile_pool(name="poolp", bufs=2))
    ptmp = ctx.enter_context(tc.tile_pool(name="ptmp", bufs=1))
    gtp = ctx.enter_context(tc.tile_pool(name="gtp", bufs=2))
    fxp = ctx.enter_context(tc.tile_pool(name="fxp", bufs=2))
    znp = ctx.enter_context(tc.tile_pool(name="znp", bufs=8))
    ztp = ctx.enter_context(tc.tile_pool(name="ztp", bufs=2))
    wzp = ctx.enter_context(tc.tile_pool(name="wzp", bufs=16))
    wnp = ctx.enter_context(tc.tile_pool(name="wnp", bufs=6))
    outp = ctx.enter_context(tc.tile_pool(name="outp", bufs=8))
    psf = ctx.enter_context(tc.tile_pool(name="psf", bufs=2, space="PSUM"))
    pbank = ctx.enter_context(tc.tile_pool(name="pbank", bufs=3, space="PSUM"))
    pzz = ctx.enter_context(tc.tile_pool(name="pzz", bufs=1, space="PSUM"))
    dram = ctx.enter_context(tc.tile_pool(name="dram", bufs=1, space="DRAM"))

    # ---- constants / weights: two packed DMAs to keep sync fan-in tiny ----
    # wpack (128, 1152) bf16:
    #   [twT(2x128) pwT(2x128) gwT(2x128) wwT(256) identb(128)]
    wp_s = consts.tile([128, 1152], BF16)
    nc.sync.dma_start(out=wp_s, in_=wpack)
    tw_s = wp_s[:, 0:256].rearrange("p (k c) -> p k c", k=2)
    pw_s = wp_s[:, 256:512].rearrange("p (k c) -> p k c", k=2)
    gw_s = wp_s[:, 512:768].rearrange("p (k c) -> p k c", k=2)
    ww_s = wp_s[:, 768:1024]
    identb_s = wp_s[:, 1024:1152]
    # vpack (128, 261) f32: [tb, gamma(2), beta(2), wraw(2x128), wwT(256)]
    vp_s = consts.tile([128, 262], F32)
    nc.sync.dma_start(out=vp_s, in_=vpack)
    tb_s = vp_s[:, 0:1]
    gamma_s = vp_s[:, 1:3]
    beta_s = vp_s[:, 3:5]
    wraw_s = vp_s[:, 5:261].rearrange("p (k c) -> p k c", k=2)
    eps_s = vp_s[:, 261:262]
    wwr_s = consts.tile([128, 256], F32R)
    nc.sync.dma_start(out=wwr_s, in_=io["wwr"])

    acc_s = consts.tile([128, BLOC * NMC], F32)       # per (b, mc) z sums
    zz_ps = pzz.tile([128, 128], F32)                 # z Gram matrix accumulator

    zz_n = [0]
    zz_last = BLOC * NMC * 4 - 1                      # 64 accumulated Gram matmuls

    # ---------------- input DMAs for both items, issued upfront -------------
    # y[b] first (the attention loop needs the full phi before it can start),
    # then x[b]; item 1's loads stream in behind item 0's while item 0
    # computes, so the item transition pays no DMA latency.
    x_tiles = {}
    y_tiles_all = {}
    for b in range(BLOC):
        ys = y[b].rearrange("(k p) m -> p k m", p=128)
        y_tiles_all[b] = []
        for q in range(4):
            yr = yin.tile([128, 2, 2 * MC], BF16, tag="yin", name=f"y_{b}_{q}")
            nc.sync.dma_start(out=yr, in_=ys[:, :, q * 2 * MC:(q + 1) * 2 * MC])
            y_tiles_all[b].append(yr)

        x_t = xfp.tile([128, 2, N], BF16, tag="xf", name=f"x_{b}")
        x_tiles[b] = x_t
        xs = x[b].rearrange("(k p) m -> p k m", p=128)
        for q in range(4):
            qs = slice(q * (N // 4), (q + 1) * (N // 4))
            nc.sync.dma_start(out=x_t[:, :, qs], in_=xs[:, :, qs])

    zt_tiles = {}
    phi_tiles = {}
    gt_tiles = {}
    theta_tiles = {}

    def conv_phase(b):
        x_t = x_tiles[b]
        y_tiles = y_tiles_all[b]

        # ---------------- phi/g convs + 2x2 maxpool ----------------
        # pooled tensors: (128ci, 32ph, 32pw)
        phi_p = poolp.tile([128, 32, 32], F32R, tag="phi_p")
        g_p = poolp.tile([128, 32, 32], BF16, tag="g_p")
        for mc in range(NMC):
            yr = y_tiles[mc // 2]
            half = slice((mc % 2) * MC, (mc % 2 + 1) * MC)
            for which, w_s, dst in (("phi", pw_s, phi_p), ("g", gw_s, g_p)):
                cps = pbank.tile([128, MC], F32, tag="bank", name=f"cps_{which}")
                nc.tensor.matmul(cps, w_s[:, 0, :], yr[:, 0, half], start=True, stop=False)
                nc.tensor.matmul(cps, w_s[:, 1, :], yr[:, 1, half], start=False, stop=True)
                # 2x2 maxpool in one reduce: (128, 4ph, 32pw, 2hh, 2ww) -> XY
                v = cps.rearrange("p (ph hh pw ww) -> p ph pw hh ww", ph=4, hh=2, ww=2)
                nc.vector.tensor_reduce(
                    out=dst[:, mc * 4:(mc + 1) * 4, :], in_=v, axis=AX.XY, op=ALU.max,
                )

        # ---------------- theta conv: (128ci, 4096) ----------------
        theta = thp.tile([128, N], F32R, tag="theta")
        for mc in range(NMC):
            ms = slice(mc * MC, (mc + 1) * MC)
            tps = pbank.tile([128, MC], F32, tag="bank", name="tps")
            nc.tensor.matmul(tps, tw_s[:, 0, :], x_t[:, 0, ms], start=True, stop=False)
            nc.tensor.matmul(tps, tw_s[:, 1, :], x_t[:, 1, ms], start=False, stop=True)
            nc.scalar.activation(theta[:, ms], tps, ACT.Identity, bias=tb_s, scale=1.0)

        # ---------------- gT tiles with [ones | zeros] pad columns ----------
        # gt: (128 pooled, 8 pchunk, 132) ; [:, :, 0:128]=g^T, col 128=1, rest 0
        gt = gtp.tile([128, 8, GW], BF16, tag="gt")
        nc.sync.dma_start(out=gt[:, :, 128:GW], in_=gpad)
        g_flat = g_p.rearrange("p a b -> p (a b)")
        for half in range(2):
            gtps = pbank.tile([128, 4, 128], BF16, tag="bank", name="gtps")
            for j in range(4):
                pch = half * 4 + j
                nc.tensor.transpose(
                    gtps[:, j, :], g_flat[:, pch * 128:(pch + 1) * 128],
                    identb_s,
                )
            nc.vector.tensor_copy(out=gt[:, half * 4:(half + 1) * 4, 0:128], in_=gtps)

        phi_tiles[b] = phi_p
        gt_tiles[b] = gt
        theta_tiles[b] = theta

    # ---------------- attention per item / m-chunk ----------------
    # item b+1's conv phase is injected midway through item b's attention so
    # neither the PE nor the ACT exp chain drains at the item boundary, while
    # attention 0 still starts as soon as item 0's inputs land.
    def attention_phase(b, inject_mc=None, inject_fn=None):
        theta = theta_tiles[b]
        zt = ztp.tile([128, N], BF16, tag="zt", name=f"zt_{b}")
        zt_tiles[b] = zt
        phi_flat = phi_tiles[b].rearrange("p a b -> p (a b)")
        gt = gt_tiles[b]
        for mc in range(NMC):
            if mc == inject_mc and inject_fn is not None:
                inject_fn()
            ms = slice(mc * MC, (mc + 1) * MC)
            # fT tiles: (128 pooled, 512 m) for each of 8 pooled chunks; exp on ACT
            fexp = fxp.tile([128, 8, MC], BF16, tag="fexp")
            for half in range(4):
                fps = psf.tile([128, 2, MC], F32, tag="f")
                for i in range(2):
                    pch = half * 2 + i
                    nc.tensor.matmul(
                        fps[:, i, :],
                        phi_flat[:, pch * 128:(pch + 1) * 128],
                        theta[:, ms],
                        start=True, stop=True,
                    )
                nc.scalar.activation(fexp[:, 2 * half:2 * half + 2, :], fps, ACT.Exp)

            # z matmuls: out (128 m, [z | s | junk]) accumulated over 8 pooled
            # chunks; two m-subtiles per PSUM bank tile
            tp = pbank.tile([128, 4, 128], BF16, tag="bank", name="tp")
            for j2 in range(2):
                zb = pbank.tile([128, 512], F32, tag="bank", name="zb")
                for i in range(2):
                    sub = j2 * 2 + i
                    for pch in range(8):
                        nc.tensor.matmul(
                            zb[:, i * 256:i * 256 + GW],
                            fexp[:, pch, sub * 128:(sub + 1) * 128],
                            gt[:, pch, :],
                            start=(pch == 0), stop=(pch == 7),
                        )
                zb2 = zb.rearrange("p (i c) -> p i c", i=2)
                rc = ptmp.tile([128, 2], F32, tag="rc", bufs=6)
                nc.vector.reciprocal(rc, zb2[:, :, 128])
                zn2 = znp.tile([128, 2, 128], BF16, tag="zn")
                nc.vector.tensor_tensor(
                    zn2, zb2[:, :, 0:128],
                    rc[:, :, None].to_broadcast((128, 2, 128)), ALU.mult,
                )
                for i in range(2):
                    sub = j2 * 2 + i
                    nc.tensor.transpose(tp[:, sub, :], zn2[:, i, :], identb_s)
                    # Gram accumulation for BN variance: ZZ += z_m^T z_m
                    nc.tensor.matmul(
                        zz_ps, zn2[:, i, :], zn2[:, i, :],
                        start=(zz_n[0] == 0), stop=(zz_n[0] == zz_last),
                        skip_group_check=True,
                    )
                    zz_n[0] += 1
            # zt copy doubles as the per-chunk z column-sum for the BN mean
            nc.vector.tensor_scalar(
                zt[:, ms], tp.rearrange("p a b -> p (a b)"), 1.0, 0.0,
                ALU.mult, ALU.add, accum_out=acc_s[:, b * NMC + mc:b * NMC + mc + 1],
            )


    conv_phase(0)
    conv_phase(1)
    attention_phase(0)
    attention_phase(1)

    # ---------------- global BN stats via AllReduce ----------------
    # ls[:, cc] = sum(wz), ls[:, 2+cc] = sum(wz^2), both derived from z
    ls = consts.tile([128, 4], F32)
    sumz = consts.tile([128, 2], F32R)
    with nc.allow_low_precision(reason="f32r is full-width f32 storage"):
        nc.vector.reduce_sum(out=sumz[:, 0:1], in_=acc_s, axis=AX.X)
        nc.vector.reduce_sum(out=sumz[:, 1:2], in_=acc_s[:, 0:1], axis=AX.X)
    # sum(wz)_c = W[c,:] @ sumz ; sum(wz^2)_c = diag(W ZZ W^T)
    zz_s = consts.tile([128, 128], F32R)
    nc.vector.tensor_copy(out=zz_s, in_=zz_ps)
    for cc in range(2):
        s1_ps = pbank.tile([128, 2], F32, tag="bank", name="s1_ps")
        nc.tensor.matmul(s1_ps, wwr_s[:, cc * 128:(cc + 1) * 128], sumz,
                         start=True, stop=True)
        nc.vector.tensor_copy(out=ls[:, cc:cc + 1], in_=s1_ps[:, 0:1])
        u_ps = pbank.tile([128, 128], F32, tag="bank", name="u_ps")
        nc.tensor.matmul(u_ps, wwr_s[:, cc * 128:(cc + 1) * 128], zz_s,
                         start=True, stop=True)
        qjunk = ptmp.tile([128, 128], F32, tag="qjunk", bufs=1)
        nc.vector.scalar_tensor_tensor(
            qjunk, u_ps, 1.0, wraw_s[:, cc, :], ALU.mult, ALU.mult,
            accum_out=ls[:, 2 + cc:3 + cc],
        )

    cc_in = dram.tile([128, 4], F32)
    cc_out = dram.tile([128, 4], F32)
    nc.sync.dma_start(out=cc_in, in_=ls)

    if io.get("single_core_sim"):
        # stand-in for the AllReduce so TimelineSim (single-core) can run
        nc.sync.dma_start(out=cc_out, in_=cc_in)
    else:
        nc.gpsimd.collective_compute(
            "AllReduce", ALU.add,
            replica_groups=[list(range(NCORES))],
            ins=[cc_in.opt()], outs=[cc_out.opt()],
        )
    gs = consts.tile([128, 4], F32)
    nc.sync.dma_start(out=gs, in_=cc_out)

    inv = 1.0 / COUNT
    st4 = consts.tile([128, 4], F32)
    nc.vector.tensor_scalar(st4, gs, inv, None, ALU.mult)
    mean = st4[:, 0:2]
    e2 = st4[:, 2:4]
    msq = consts.tile([128, 2], F32)
    nc.vector.tensor_mul(msq, mean, mean)
    u = consts.tile([128, 2], F32)
    nc.vector.tensor_sub(u, e2, msq)
    nc.vector.tensor_scalar(u, u, EPS, None, ALU.add)
    # rsqrt(u) = exp(-0.5*ln(u)) -- one ACT table switch, costs ~1.3us
    y0 = consts.tile([128, 2], F32)
    nc.scalar.activation(y0, u, ACT.Ln)
    r0 = consts.tile([128, 2], F32)
    nc.scalar.activation(r0, y0, ACT.Exp, scale=-0.5)
    a_s = consts.tile([128, 2], F32)
    nc.vector.tensor_mul(a_s, r0, gamma_s)
    nb = consts.tile([128, 2], F32)
    nc.vector.tensor_mul(nb, mean, a_s)
    nc.vector.tensor_sub(nb, beta_s, nb)

    # ---------------- W conv + normalize + residual + store ----------------
    for b in range(BLOC):
        x_t = x_tiles[b]
        zt = zt_tiles[b]
        for mc in range(NMC):
            ms = slice(mc * MC, (mc + 1) * MC)
            for cc in range(2):
                csl = slice(cc * 128, (cc + 1) * 128)
                wb = pbank.tile([128, MC], F32, tag="bank", name="wb")
                nc.tensor.matmul(
                    wb, ww_s[:, cc * 128:(cc + 1) * 128], zt[:, ms],
                    start=True, stop=True,
                )
                # normalize on ACT: wn = wz*a + (beta - mean*a)
                wn = wnp.tile([128, MC], BF16, tag="wn")
                nc.scalar.activation(
                    wn, wb, ACT.Identity,
                    bias=nb[:, cc:cc + 1], scale=a_s[:, cc:cc + 1],
                )
                ot = outp.tile([128, MC], F32, tag="ot")
                eng = nc.vector if (mc + cc) % 2 == 0 else nc.gpsimd
                eng.tensor_add(ot, wn, x_t[:, cc, ms])
                nc.sync.dma_start(out=out[b, csl, ms], in_=ot)


def make_io(nc):
    return {
        "x": nc.dram_tensor("x", [BLOC, C, N], BF16, kind="ExternalInput").ap(),
        "y": nc.dram_tensor("y", [BLOC, C, N], BF16, kind="ExternalInput").ap(),
        "wpack": nc.dram_tensor("wpack", [128, 1152], BF16, kind="ExternalInput").ap(),
        "vpack": nc.dram_tensor("vpack", [128, 262], F32, kind="ExternalInput").ap(),
        "wwr": nc.dram_tensor("wwr", [128, 256], mybir.dt.float32r, kind="ExternalInput").ap(),
        "gpad": nc.dram_tensor("gpad", [128, 8, 4], BF16, kind="ExternalInput").ap(),
        "out": nc.dram_tensor("out", [BLOC, C, N], F32, kind="ExternalOutput").ap(),
    }


_CACHE = {}


def _get_program():
    if "nc" in _CACHE:
        return _CACHE["nc"], _CACHE["io"]
    nc = bacc.Bacc(
        "TRN2", target_bir_lowering=False, debug=False,
        enable_asserts=False, num_devices=NCORES,
    )
    io = make_io(nc)
    from contextlib import ExitStack
    with tile.TileContext(nc) as tc:
        with ExitStack() as ctx:
            io["ctx"] = ctx
            build_body(tc, io)
    nc.compile()
    _CACHE["nc"] = nc
    _CACHE["io"] = io
    return nc, io


def kernel(x, y, theta_w, theta_b, phi_w, phi_b, g_w, g_b, W_w, W_b,
           bn_gamma, bn_beta, _trace=False, **_unused):
    x = np.asarray(x, dtype=np.float32).reshape(B, C, N).astype(ml_bf16)
    y = np.asarray(y, dtype=np.float32).reshape(B, C, N).astype(ml_bf16)

    def chunked(wT):
        # (C, CI) -> (128, 2, CI): [p, k, ci] = wT[k*128+p, ci]
        return np.asarray(wT, np.float32).reshape(2, 128, CI).transpose(1, 0, 2)

    tw = chunked(np.asarray(theta_w, np.float32).T)
    pw = chunked(np.asarray(phi_w, np.float32).T)
    gw = chunked(np.asarray(g_w, np.float32).T)
    ww = np.asarray(W_w, np.float32).T                             # (CI, C)
    wraw = chunked(np.asarray(W_w, np.float32))                    # c-part layout
    ident = np.eye(128, dtype=np.float32)
    wpack = np.ascontiguousarray(np.concatenate([
        tw.reshape(128, 256), pw.reshape(128, 256), gw.reshape(128, 256),
        ww, ident], axis=1).astype(ml_bf16))
    tb = np.asarray(theta_b, np.float32).reshape(CI, 1)
    gamma = np.asarray(bn_gamma, np.float32).reshape(2, 128).T
    beta = np.asarray(bn_beta, np.float32).reshape(2, 128).T
    vpack = np.ascontiguousarray(np.concatenate(
        [tb, gamma, beta, wraw.reshape(128, 256),
         np.full((128, 1), EPS, np.float32)], axis=1))
    wwr = np.ascontiguousarray(ww)
    gpad = np.zeros((128, 8, 4), ml_bf16)
    gpad[:, :, 0] = 1.0
    # phi_b, g_b, W_b intentionally unused: softmax-invariant / cancelled by BN.

    nc, _ = _get_program()
    in_maps = []
    for k in range(NCORES):
        in_maps.append({
            "x": np.ascontiguousarray(x[k * BLOC:(k + 1) * BLOC]),
            "y": np.ascontiguousarray(y[k * BLOC:(k + 1) * BLOC]),
            "wpack": wpack, "vpack": vpack, "wwr": wwr, "gpad": gpad,
        })
    res = run_bass_kernel_spmd(nc, in_maps, core_ids=list(range(NCORES)), trace=_trace)
    out = np.concatenate([r_["out"] for r_ in res.results], axis=0)
    if _trace:
        _CACHE["last_results"] = res
    return out.reshape(B, C, 64, 64)


# revision 36
# speedup vs baseline: 1.0466x; 1.0130x over previous
# Trainium2 Bass kernel for the non-local attention block (nn_DRAL_88476326297980).
#
# Reference computation (per batch b):
#   theta = theta_w @ x_b + theta_b            (CI=128, N=4096)
#   phi   = maxpool2x2(phi_w @ y_b + phi_b)    (CI=128, P=1024)
#   g     = maxpool2x2(g_w  @ y_b + g_b)       (CI=128, P=1024)
#   f     = theta^T @ phi                      (N, P)
#   fdiv  = softmax(f, axis=P)
#   z     = fdiv @ g^T                         (N, CI)
#   wz    = W_w @ z^T + W_b                    (C=256, N)
#   out   = BN(wz over all b,n) + x            (training-mode batch stats)
#
# Sharding: data-parallel over batch, 2 batches per core, 8 cores.
# BN batch statistics are combined with a tiny (128x4) AllReduce.
#
# Math simplifications used (exact, not approximations):
#  - phi_b adds a per-row constant to f -> softmax-invariant -> dropped.
#  - g_b adds a per-CI constant to z (softmax weights sum to 1) -> shifts wz
#    per-channel -> cancelled by the BN mean subtraction -> dropped.
#  - W_b shifts wz per-channel -> cancelled by BN mean subtraction -> dropped.
#  - BN statistics are computed from z (before the W conv):
#      sum(wz)_c  = W @ sum_m(z_m)     (sum_m z from accum on the zt copies)
#      sum(wz^2)_c = diag(W ZZ W^T)    (ZZ = z Gram matrix, accumulated in PSUM)
#    so the AllReduce fires right after the last attention tile and the
#    W conv + normalize + residual + store run as one fused pipeline.
#
# Layout choices:
#  - everything on the PE runs in bf16 (full rate at any tile size; f32r pays
#    4x below 256 output cols, which hit the old Gram/transpose path).
#  - f is computed TRANSPOSED (fT: pooled dim on partitions, n on free) so both
#    attention matmuls contract over the partition dim with no transposes of f.
#  - softmax denominators come from an extra all-ones column appended to the
#    g^T tiles: the z-matmul then yields [z_unnorm | s | junk] in one PSUM
#    accumulation group.
#  - x and y are loaded as bf16 (host-converted), halving input HBM traffic.

import numpy as np
from ml_dtypes import bfloat16 as ml_bf16

import concourse.bass as bass
import concourse.mybir as mybir
import concourse.tile as tile
from concourse import bacc
from concourse.bass_utils import run_bass_kernel_spmd

F32 = mybir.dt.float32
F32R = mybir.dt.float32r
BF16 = mybir.dt.bfloat16
ALU = mybir.AluOpType
ACT = mybir.ActivationFunctionType
AX = mybir.AxisListType

NCORES = 8
B = 16
BLOC = B // NCORES          # 2 batches per core
C = 256                     # in channels
CI = 128                    # inter channels
N = 4096                    # h*w
MC = 512                    # m-chunk (columns per matmul)
NMC = N // MC               # 8
GW = 130                    # z-matmul output width: 128 z cols + s + 1 junk
EPS = 1e-5
COUNT = B * N               # BN sample count per channel


def build_body(tc, io):
    nc = tc.nc
    x, y, wpack, vpack, gpad, out = (
        io["x"], io["y"], io["wpack"], io["vpack"], io["gpad"], io["out"],
    )

    ctx = io["ctx"]
    consts = ctx.enter_context(tc.tile_pool(name="consts", bufs=1))
    xfp = ctx.enter_context(tc.tile_pool(name="xfp", bufs=2))
    yin = ctx.enter_context(tc.tile_pool(name="yin", bufs=8))
    thp = ctx.enter_context(tc.tile_pool(name="thp", bufs=2))
    poolp = ctx.enter_context(tc.tile_pool(name="poolp", bufs=2))
    ptmp = ctx.enter_context(tc.tile_pool(name="ptmp", bufs=1))
    gtp = ctx.enter_context(tc.tile_pool(name="gtp", bufs=2))
    fxp = ctx.enter_context(tc.tile_pool(name="fxp", bufs=2))
    znp = ctx.enter_context(tc.tile_pool(name="znp", bufs=8))
    ztp = ctx.enter_context(tc.tile_pool(name="ztp", bufs=2))
    wzp = ctx.enter_context(tc.tile_pool(name="wzp", bufs=16))
    wnp = ctx.enter_context(tc.tile_pool(name="wnp", bufs=6))
    outp = ctx.enter_context(tc.tile_pool(name="outp", bufs=8))
    psf = ctx.enter_context(tc.tile_pool(name="psf", bufs=2, space="PSUM"))
    pbank = ctx.enter_context(tc.tile_pool(name="pbank", bufs=3, space="PSUM"))
    pzz = ctx.enter_context(tc.tile_pool(name="pzz", bufs=1, space="PSUM"))
    dram = ctx.enter_context(tc.tile_pool(name="dram", bufs=1, space="DRAM"))

    # ---- constants / weights: two packed DMAs to keep sync fan-in tiny ----
    # wpack (128, 1152) bf16:
    #   [twT(2x128) pwT(2x128) gwT(2x128) wwT(256) identb(128)]
    wp_s = consts.tile([128, 1152], BF16)
    nc.sync.dma_start(out=wp_s, in_=wpack)
    tw_s = wp_s[:, 0:256].rearrange("p (k c) -> p k c", k=2)
    pw_s = wp_s[:, 256:512].rearrange("p (k c) -> p k c", k=2)
    gw_s = wp_s[:, 512:768].rearrange("p (k c) -> p k c", k=2)
    ww_s = wp_s[:, 768:1024]
    identb_s = wp_s[:, 1024:1152]
    # vpack (128, 261) f32: [tb, gamma(2), beta(2), wraw(2x128), wwT(256)]
    vp_s = consts.tile([128, 262], F32)
    nc.sync.dma_start(out=vp_s, in_=vpack)
    tb_s = vp_s[:, 0:1]
    gamma_s = vp_s[:, 1:3]
    beta_s = vp_s[:, 3:5]
    wraw_s = vp_s[:, 5:261].rearrange("p (k c) -> p k c", k=2)
    eps_s = vp_s[:, 261:262]
    wwr_s = consts.tile([128, 256], F32R)
    nc.sync.dma_start(out=wwr_s, in_=io["wwr"])

    acc_s = consts.tile([128, BLOC * NMC], F32)       # per (b, mc) z sums
    zz_ps = pzz.tile([128, 128], F32)                 # z Gram matrix accumulator

    zz_n = [0]
    zz_last = BLOC * NMC * 4 - 1                      # 64 accumulated Gram matmuls

    # ---------------- input DMAs for both items, issued upfront -------------
    # y[b] first (the attention loop needs the full phi before it can start),
    # then x[b]; item 1's loads stream in behind item 0's while item 0
    # computes, so the item transition pays no DMA latency.
    x_tiles = {}
    y_tiles_all = {}
    for b in range(BLOC):
        ys = y[b].rearrange("(k p) m -> p k m", p=128)
        y_tiles_all[b] = []
        for q in range(4):
            yr = yin.tile([128, 2, 2 * MC], BF16, tag="yin", name=f"y_{b}_{q}")
            nc.sync.dma_start(out=yr, in_=ys[:, :, q * 2 * MC:(q + 1) * 2 * MC])
            y_tiles_all[b].append(yr)

        x_t = xfp.tile([128, 2, N], BF16, tag="xf", name=f"x_{b}")
        x_tiles[b] = x_t
        xs = x[b].rearrange("(k p) m -> p k m", p=128)
        for q in range(4):
            qs = slice(q * (N // 4), (q + 1) * (N // 4))
            nc.sync.dma_start(out=x_t[:, :, qs], in_=xs[:, :, qs])

    zt_tiles = {}
    phi_tiles = {}
    gt_tiles = {}
    theta_tiles = {}

    def theta_phase(b):
        x_t = x_tiles[b]
        # ---------------- theta conv: (128ci, 4096) ----------------
        theta = thp.tile([128, N], F32R, tag="theta")
        for mc in range(NMC):
            ms = slice(mc * MC, (mc + 1) * MC)
            tps = pbank.tile([128, MC], F32, tag="bank", name="tps")
            nc.tensor.matmul(tps, tw_s[:, 0, :], x_t[:, 0, ms], start=True, stop=False)
            nc.tensor.matmul(tps, tw_s[:, 1, :], x_t[:, 1, ms], start=False, stop=True)
            nc.scalar.activation(theta[:, ms], tps, ACT.Identity, bias=tb_s, scale=1.0)
        theta_tiles[b] = theta

    def conv_phase(b, with_theta=True):
        x_t = x_tiles[b]
        y_tiles = y_tiles_all[b]

        # ---------------- phi/g convs + 2x2 maxpool ----------------
        # pooled tensors: (128ci, 32ph, 32pw)
        phi_p = poolp.tile([128, 32, 32], F32R, tag="phi_p")
        g_p = poolp.tile([128, 32, 32], BF16, tag="g_p")
        for mc in range(NMC):
            yr = y_tiles[mc // 2]
            half = slice((mc % 2) * MC, (mc % 2 + 1) * MC)
            for which, w_s, dst in (("phi", pw_s, phi_p), ("g", gw_s, g_p)):
                cps = pbank.tile([128, MC], F32, tag="bank", name=f"cps_{which}")
                nc.tensor.matmul(cps, w_s[:, 0, :], yr[:, 0, half], start=True, stop=False)
                nc.tensor.matmul(cps, w_s[:, 1, :], yr[:, 1, half], start=False, stop=True)
                # 2x2 maxpool in one reduce: (128, 4ph, 32pw, 2hh, 2ww) -> XY
                v = cps.rearrange("p (ph hh pw ww) -> p ph pw hh ww", ph=4, hh=2, ww=2)
                nc.vector.tensor_reduce(
                    out=dst[:, mc * 4:(mc + 1) * 4, :], in_=v, axis=AX.XY, op=ALU.max,
                )

        if with_theta:
            theta_phase(b)

        # ---------------- gT tiles with [ones | zeros] pad columns ----------
        # gt: (128 pooled, 8 pchunk, 132) ; [:, :, 0:128]=g^T, col 128=1, rest 0
        gt = gtp.tile([128, 8, GW], BF16, tag="gt")
        nc.sync.dma_start(out=gt[:, :, 128:GW], in_=gpad[:, :, 0:GW - 128])
        g_flat = g_p.rearrange("p a b -> p (a b)")
        for half in range(2):
            gtps = pbank.tile([128, 4, 128], BF16, tag="bank", name="gtps")
            for j in range(4):
                pch = half * 4 + j
                nc.tensor.transpose(
                    gtps[:, j, :], g_flat[:, pch * 128:(pch + 1) * 128],
                    identb_s,
                )
            nc.vector.tensor_copy(out=gt[:, half * 4:(half + 1) * 4, 0:128], in_=gtps)

        phi_tiles[b] = phi_p
        gt_tiles[b] = gt

    # ---------------- attention per item / m-chunk ----------------
    # item b+1's conv phase is injected midway through item b's attention so
    # neither the PE nor the ACT exp chain drains at the item boundary, while
    # attention 0 still starts as soon as item 0's inputs land.
    def attention_phase(b, inject_mc=None, inject_fn=None):
        theta = theta_tiles[b]
        zt = ztp.tile([128, N], BF16, tag="zt", name=f"zt_{b}")
        zt_tiles[b] = zt
        phi_flat = phi_tiles[b].rearrange("p a b -> p (a b)")
        gt = gt_tiles[b]
        for mc in range(NMC):
            if mc == inject_mc and inject_fn is not None:
                inject_fn()
            ms = slice(mc * MC, (mc + 1) * MC)
            # fT tiles: (128 pooled, 512 m) for each of 8 pooled chunks; exp on ACT
            fexp = fxp.tile([128, 8, MC], BF16, tag="fexp")
            for half in range(4):
                fps = psf.tile([128, 2, MC], F32, tag="f")
                for i in range(2):
                    pch = half * 2 + i
                    nc.tensor.matmul(
                        fps[:, i, :],
                        phi_flat[:, pch * 128:(pch + 1) * 128],
                        theta[:, ms],
                        start=True, stop=True,
                    )
                nc.scalar.activation(fexp[:, 2 * half:2 * half + 2, :], fps, ACT.Exp)

            # z matmuls: out (128 m, [z | s | junk]) accumulated over 8 pooled
            # chunks; two m-subtiles per PSUM bank tile
            tp = pbank.tile([128, 4, 128], BF16, tag="bank", name="tp")
            for j2 in range(2):
                zb = pbank.tile([128, 512], F32, tag="bank", name="zb")
                for i in range(2):
                    sub = j2 * 2 + i
                    for pch in range(8):
                        nc.tensor.matmul(
                            zb[:, i * 256:i * 256 + GW],
                            fexp[:, pch, sub * 128:(sub + 1) * 128],
                            gt[:, pch, :],
                            start=(pch == 0), stop=(pch == 7),
                        )
                zb2 = zb.rearrange("p (i c) -> p i c", i=2)
                rc = ptmp.tile([128, 2], F32, tag="rc", bufs=6)
                nc.vector.reciprocal(rc, zb2[:, :, 128])
                zn2 = znp.tile([128, 2, 128], BF16, tag="zn")
                nc.vector.tensor_tensor(
                    zn2, zb2[:, :, 0:128],
                    rc[:, :, None].to_broadcast((128, 2, 128)), ALU.mult,
                )
                for i in range(2):
                    sub = j2 * 2 + i
                    nc.tensor.transpose(tp[:, sub, :], zn2[:, i, :], identb_s)
                    # Gram accumulation for BN variance: ZZ += z_m^T z_m
                    nc.tensor.matmul(
                        zz_ps, zn2[:, i, :], zn2[:, i, :],
                        start=(zz_n[0] == 0), stop=(zz_n[0] == zz_last),
                        skip_group_check=True,
                    )
                    zz_n[0] += 1
            # zt copy doubles as the per-chunk z column-sum for the BN mean
            nc.vector.tensor_scalar(
                zt[:, ms], tp.rearrange("p a b -> p (a b)"), 1.0, 0.0,
                ALU.mult, ALU.add, accum_out=acc_s[:, b * NMC + mc:b * NMC + mc + 1],
            )


    conv_phase(0)
    conv_phase(1)
    attention_phase(0)
    attention_phase(1)

    # ---------------- global BN stats via AllReduce ----------------
    # ls[:, cc] = sum(wz), ls[:, 2+cc] = sum(wz^2), both derived from z
    ls = consts.tile([128, 4], F32)
    sumz = consts.tile([128, 2], F32R)
    with nc.allow_low_precision(reason="f32r is full-width f32 storage"):
        nc.vector.reduce_sum(out=sumz[:, 0:1], in_=acc_s, axis=AX.X)
        nc.vector.reduce_sum(out=sumz[:, 1:2], in_=acc_s[:, 0:1], axis=AX.X)
    # sum(wz)_c = W[c,:] @ sumz ; sum(wz^2)_c = diag(W ZZ W^T)
    zz_s = consts.tile([128, 128], F32R)
    nc.vector.tensor_copy(out=zz_s, in_=zz_ps)
    for cc in range(2):
        s1_ps = pbank.tile([128, 2], F32, tag="bank", name="s1_ps")
        nc.tensor.matmul(s1_ps, wwr_s[:, cc * 128:(cc + 1) * 128], sumz,
                         start=True, stop=True)
        nc.vector.tensor_copy(out=ls[:, cc:cc + 1], in_=s1_ps[:, 0:1])
        u_ps = pbank.tile([128, 128], F32, tag="bank", name="u_ps")
        nc.tensor.matmul(u_ps, wwr_s[:, cc * 128:(cc + 1) * 128], zz_s,
                         start=True, stop=True)
        qjunk = ptmp.tile([128, 128], F32, tag="qjunk", bufs=1)
        nc.vector.scalar_tensor_tensor(
            qjunk, u_ps, 1.0, wraw_s[:, cc, :], ALU.mult, ALU.mult,
            accum_out=ls[:, 2 + cc:3 + cc],
        )

    cc_in = dram.tile([128, 4], F32)
    cc_out = dram.tile([128, 4], F32)
    nc.sync.dma_start(out=cc_in, in_=ls)

    if io.get("single_core_sim"):
        # stand-in for the AllReduce so TimelineSim (single-core) can run
        nc.sync.dma_start(out=cc_out, in_=cc_in)
    else:
        nc.gpsimd.collective_compute(
            "AllReduce", ALU.add,
            replica_groups=[list(range(NCORES))],
            ins=[cc_in.opt()], outs=[cc_out.opt()],
        )
    gs = consts.tile([128, 4], F32)
    nc.sync.dma_start(out=gs, in_=cc_out)

    inv = 1.0 / COUNT
    st4 = consts.tile([128, 4], F32)
    nc.vector.tensor_scalar(st4, gs, inv, None, ALU.mult)
    mean = st4[:, 0:2]
    e2 = st4[:, 2:4]
    msq = consts.tile([128, 2], F32)
    nc.vector.tensor_mul(msq, mean, mean)
    u = consts.tile([128, 2], F32)
    nc.vector.tensor_sub(u, e2, msq)
    nc.vector.tensor_scalar(u, u, EPS, None, ALU.add)
    # rsqrt(u) = exp(-0.5*ln(u)) -- one ACT table switch, costs ~1.3us
    y0 = consts.tile([128, 2], F32)
    nc.scalar.activation(y0, u, ACT.Ln)
    r0 = consts.tile([128, 2], F32)
    nc.scalar.activation(r0, y0, ACT.Exp, scale=-0.5)
    a_s = consts.tile([128, 2], F32)
    nc.vector.tensor_mul(a_s, r0, gamma_s)
    nb = consts.tile([128, 2], F32)
    nc.vector.tensor_mul(nb, mean, a_s)
    nc.vector.tensor_sub(nb, beta_s, nb)

    # ---------------- W conv + normalize + residual + store ----------------
    for b in range(BLOC):
        x_t = x_tiles[b]
        zt = zt_tiles[b]
        for mc in range(NMC):
            ms = slice(mc * MC, (mc + 1) * MC)
            for cc in range(2):
                csl = slice(cc * 128, (cc + 1) * 128)
                wb = pbank.tile([128, MC], F32, tag="bank", name="wb")
                nc.tensor.matmul(
                    wb, ww_s[:, cc * 128:(cc + 1) * 128], zt[:, ms],
                    start=True, stop=True,
                )
                # normalize on ACT: wn = wz*a + (beta - mean*a)
                wn = wnp.tile([128, MC], BF16, tag="wn")
                nc.scalar.activation(
                    wn, wb, ACT.Identity,
                    bias=nb[:, cc:cc + 1], scale=a_s[:, cc:cc + 1],
                )
                ot = outp.tile([128, MC], F32, tag="ot")
                eng = nc.gpsimd if (2 * mc + cc) % 4 == 3 else nc.vector
                eng.tensor_add(ot, wn, x_t[:, cc, ms])
                nc.sync.dma_start(out=out[b, csl, ms], in_=ot)


def make_io(nc):
    return {
        "x": nc.dram_tensor("x", [BLOC, C, N], BF16, kind="ExternalInput").ap(),
        "y": nc.dram_tensor("y", [BLOC, C, N], BF16, kind="ExternalInput").ap(),
        "wpack": nc.dram_tensor("wpack", [128, 1152], BF16, kind="ExternalInput").ap(),
        "vpack": nc.dram_tensor("vpack", [128, 262], F32, kind="ExternalInput").ap(),
        "wwr": nc.dram_tensor("wwr", [128, 256], mybir.dt.float32r, kind="ExternalInput").ap(),
        "gpad": nc.dram_tensor("gpad", [128, 8, 4], BF16, kind="ExternalInput").ap(),
        "out": nc.dram_tensor("out", [BLOC, C, N], F32, kind="ExternalOutput").ap(),
    }


_CACHE = {}


def _get_program():
    if "nc" in _CACHE:
        return _CACHE["nc"], _CACHE["io"]
    nc = bacc.Bacc(
        "TRN2", target_bir_lowering=False, debug=False,
        enable_asserts=False, num_devices=NCORES,
    )
    io = make_io(nc)
    from contextlib import ExitStack
    with tile.TileContext(nc) as tc:
        with ExitStack() as ctx:
            io["ctx"] = ctx
            build_body(tc, io)
    nc.compile()
    _CACHE["nc"] = nc
    _CACHE["io"] = io
    return nc, io


def kernel(x, y, theta_w, theta_b, phi_w, phi_b, g_w, g_b, W_w, W_b,
           bn_gamma, bn_beta, _trace=False, **_unused):
    x = np.asarray(x, dtype=np.float32).reshape(B, C, N).astype(ml_bf16)
    y = np.asarray(y, dtype=np.float32).reshape(B, C, N).astype(ml_bf16)

    def chunked(wT):
        # (C, CI) -> (128, 2, CI): [p, k, ci] = wT[k*128+p, ci]
        return np.asarray(wT, np.float32).reshape(2, 128, CI).transpose(1, 0, 2)

    tw = chunked(np.asarray(theta_w, np.float32).T)
    pw = chunked(np.asarray(phi_w, np.float32).T)
    gw = chunked(np.asarray(g_w, np.float32).T)
    ww = np.asarray(W_w, np.float32).T                             # (CI, C)
    wraw = chunked(np.asarray(W_w, np.float32))                    # c-part layout
    ident = np.eye(128, dtype=np.float32)
    wpack = np.ascontiguousarray(np.concatenate([
        tw.reshape(128, 256), pw.reshape(128, 256), gw.reshape(128, 256),
        ww, ident], axis=1).astype(ml_bf16))
    tb = np.asarray(theta_b, np.float32).reshape(CI, 1)
    gamma = np.asarray(bn_gamma, np.float32).reshape(2, 128).T
    beta = np.asarray(bn_beta, np.float32).reshape(2, 128).T
    vpack = np.ascontiguousarray(np.concatenate(
        [tb, gamma, beta, wraw.reshape(128, 256),
         np.full((128, 1), EPS, np.float32)], axis=1))
    wwr = np.ascontiguousarray(ww)
    gpad = np.zeros((128, 8, 4), ml_bf16)
    gpad[:, :, 0] = 1.0
    # phi_b, g_b, W_b intentionally unused: softmax-invariant / cancelled by BN.

    nc, _ = _get_program()
    in_maps = []
    for k in range(NCORES):
        in_maps.append({
            "x": np.ascontiguousarray(x[k * BLOC:(k + 1) * BLOC]),
            "y": np.ascontiguousarray(y[k * BLOC:(k + 1) * BLOC]),
            "wpack": wpack, "vpack": vpack, "wwr": wwr, "gpad": gpad,
        })
    res = run_bass_kernel_spmd(nc, in_maps, core_ids=list(range(NCORES)), trace=_trace)
    out = np.concatenate([r_["out"] for r_ in res.results], axis=0)
    if _trace:
        _CACHE["last_results"] = res
    return out.reshape(B, C, 64, 64)


# revision 40
# speedup vs baseline: 1.0564x; 1.0093x over previous
# Trainium2 Bass kernel for the non-local attention block (nn_DRAL_88476326297980).
#
# Reference computation (per batch b):
#   theta = theta_w @ x_b + theta_b            (CI=128, N=4096)
#   phi   = maxpool2x2(phi_w @ y_b + phi_b)    (CI=128, P=1024)
#   g     = maxpool2x2(g_w  @ y_b + g_b)       (CI=128, P=1024)
#   f     = theta^T @ phi                      (N, P)
#   fdiv  = softmax(f, axis=P)
#   z     = fdiv @ g^T                         (N, CI)
#   wz    = W_w @ z^T + W_b                    (C=256, N)
#   out   = BN(wz over all b,n) + x            (training-mode batch stats)
#
# Sharding: data-parallel over batch, 2 batches per core, 8 cores.
# BN batch statistics are combined with a tiny (128x4) AllReduce.
#
# Math simplifications used (exact, not approximations):
#  - phi_b adds a per-row constant to f -> softmax-invariant -> dropped.
#  - g_b adds a per-CI constant to z (softmax weights sum to 1) -> shifts wz
#    per-channel -> cancelled by the BN mean subtraction -> dropped.
#  - W_b shifts wz per-channel -> cancelled by BN mean subtraction -> dropped.
#  - BN statistics are computed from z (before the W conv):
#      sum(wz)_c  = W @ sum_m(z_m)     (sum_m z from accum on the zt copies)
#      sum(wz^2)_c = diag(W ZZ W^T)    (ZZ = z Gram matrix, accumulated in PSUM)
#    so the AllReduce fires right after the last attention tile and the
#    W conv + normalize + residual + store run as one fused pipeline.
#
# Layout choices:
#  - everything on the PE runs in bf16 (full rate at any tile size; f32r pays
#    4x below 256 output cols, which hit the old Gram/transpose path).
#  - f is computed TRANSPOSED (fT: pooled dim on partitions, n on free) so both
#    attention matmuls contract over the partition dim with no transposes of f.
#  - softmax denominators come from an extra all-ones column appended to the
#    g^T tiles: the z-matmul then yields [z_unnorm | s | junk] in one PSUM
#    accumulation group.
#  - x and y are loaded as bf16 (host-converted), halving input HBM traffic.

import numpy as np
from ml_dtypes import bfloat16 as ml_bf16

import concourse.bass as bass
import concourse.mybir as mybir
import concourse.tile as tile
from concourse import bacc
from concourse.bass_utils import run_bass_kernel_spmd

F32 = mybir.dt.float32
F32R = mybir.dt.float32r
BF16 = mybir.dt.bfloat16
ALU = mybir.AluOpType
ACT = mybir.ActivationFunctionType
AX = mybir.AxisListType

NCORES = 8
B = 16
BLOC = B // NCORES          # 2 batches per core
C = 256                     # in channels
CI = 128                    # inter channels
N = 4096                    # h*w
MC = 512                    # m-chunk (columns per matmul)
NMC = N // MC               # 8
GW = 130                    # z-matmul output width: 128 z cols + s + 1 junk
EPS = 1e-5
COUNT = B * N               # BN sample count per channel


def build_body(tc, io):
    nc = tc.nc
    x, y, wpack, vpack, gpad, out = (
        io["x"], io["y"], io["wpack"], io["vpack"], io["gpad"], io["out"],
    )

    ctx = io["ctx"]
    consts = ctx.enter_context(tc.tile_pool(name="consts", bufs=1))
    xfp = ctx.enter_context(tc.tile_pool(name="xfp", bufs=2))
    yin = ctx.enter_context(tc.tile_pool(name="yin", bufs=8))
    thp = ctx.enter_context(tc.tile_pool(name="thp", bufs=2))
    poolp = ctx.enter_context(tc.tile_pool(name="poolp", bufs=2))
    ptmp = ctx.enter_context(tc.tile_pool(name="ptmp", bufs=1))
    gtp = ctx.enter_context(tc.tile_pool(name="gtp", bufs=2))
    fxp = ctx.enter_context(tc.tile_pool(name="fxp", bufs=3))
    znp = ctx.enter_context(tc.tile_pool(name="znp", bufs=8))
    ztp = ctx.enter_context(tc.tile_pool(name="ztp", bufs=2))
    wzp = ctx.enter_context(tc.tile_pool(name="wzp", bufs=16))
    wnp = ctx.enter_context(tc.tile_pool(name="wnp", bufs=6))
    outp = ctx.enter_context(tc.tile_pool(name="outp", bufs=8))
    psf = ctx.enter_context(tc.tile_pool(name="psf", bufs=2, space="PSUM"))
    pbank = ctx.enter_context(tc.tile_pool(name="pbank", bufs=3, space="PSUM"))
    pzz = ctx.enter_context(tc.tile_pool(name="pzz", bufs=1, space="PSUM"))
    dram = ctx.enter_context(tc.tile_pool(name="dram", bufs=1, space="DRAM"))

    # ---- constants / weights: two packed DMAs to keep sync fan-in tiny ----
    # wpack (128, 1152) bf16:
    #   [twT(2x128) pwT(2x128) gwT(2x128) wwT(256) identb(128)]
    wp_s = consts.tile([128, 1152], BF16)
    nc.sync.dma_start(out=wp_s, in_=wpack)
    tw_s = wp_s[:, 0:256].rearrange("p (k c) -> p k c", k=2)
    pw_s = wp_s[:, 256:512].rearrange("p (k c) -> p k c", k=2)
    gw_s = wp_s[:, 512:768].rearrange("p (k c) -> p k c", k=2)
    ww_s = wp_s[:, 768:1024]
    identb_s = wp_s[:, 1024:1152]
    # vpack (128, 261) f32: [tb, gamma(2), beta(2), wraw(2x128), wwT(256)]
    vp_s = consts.tile([128, 262], F32)
    nc.sync.dma_start(out=vp_s, in_=vpack)
    tb_s = vp_s[:, 0:1]
    gamma_s = vp_s[:, 1:3]
    beta_s = vp_s[:, 3:5]
    wraw_s = vp_s[:, 5:261].rearrange("p (k c) -> p k c", k=2)
    eps_s = vp_s[:, 261:262]
    wwr_s = consts.tile([128, 256], F32R)
    nc.sync.dma_start(out=wwr_s, in_=io["wwr"])

    acc_s = consts.tile([128, BLOC * NMC], F32)       # per (b, mc) z sums
    zz_ps = pzz.tile([128, 128], F32)                 # z Gram matrix accumulator

    zz_n = [0]
    zz_last = BLOC * NMC * 4 - 1                      # 64 accumulated Gram matmuls

    # ---------------- input DMAs for both items, issued upfront -------------
    # y[b] first (the attention loop needs the full phi before it can start),
    # then x[b]; item 1's loads stream in behind item 0's while item 0
    # computes, so the item transition pays no DMA latency.
    x_tiles = {}
    y_tiles_all = {}
    for b in range(BLOC):
        ys = y[b].rearrange("(k p) m -> p k m", p=128)
        y_tiles_all[b] = []
        for q in range(4):
            yr = yin.tile([128, 2, 2 * MC], BF16, tag="yin", name=f"y_{b}_{q}")
            nc.sync.dma_start(out=yr, in_=ys[:, :, q * 2 * MC:(q + 1) * 2 * MC])
            y_tiles_all[b].append(yr)

        x_t = xfp.tile([128, 2, N], BF16, tag="xf", name=f"x_{b}")
        x_tiles[b] = x_t
        xs = x[b].rearrange("(k p) m -> p k m", p=128)
        for q in range(4):
            qs = slice(q * (N // 4), (q + 1) * (N // 4))
            nc.sync.dma_start(out=x_t[:, :, qs], in_=xs[:, :, qs])

    zt_tiles = {}
    phi_tiles = {}
    gt_tiles = {}
    theta_tiles = {}

    def theta_phase(b):
        x_t = x_tiles[b]
        # ---------------- theta conv: (128ci, 4096) ----------------
        theta = thp.tile([128, N], F32R, tag="theta")
        for mc in range(NMC):
            ms = slice(mc * MC, (mc + 1) * MC)
            tps = pbank.tile([128, MC], F32, tag="bank", name="tps")
            nc.tensor.matmul(tps, tw_s[:, 0, :], x_t[:, 0, ms], start=True, stop=False)
            nc.tensor.matmul(tps, tw_s[:, 1, :], x_t[:, 1, ms], start=False, stop=True)
            nc.scalar.activation(theta[:, ms], tps, ACT.Identity, bias=tb_s, scale=1.0)
        theta_tiles[b] = theta

    def conv_phase(b, with_theta=True):
        x_t = x_tiles[b]
        y_tiles = y_tiles_all[b]

        # ---------------- phi/g convs + 2x2 maxpool ----------------
        # pooled tensors: (128ci, 32ph, 32pw)
        phi_p = poolp.tile([128, 32, 32], F32R, tag="phi_p")
        g_p = poolp.tile([128, 32, 32], BF16, tag="g_p")
        for mc in range(NMC):
            yr = y_tiles[mc // 2]
            half = slice((mc % 2) * MC, (mc % 2 + 1) * MC)
            for which, w_s, dst in (("phi", pw_s, phi_p), ("g", gw_s, g_p)):
                cps = pbank.tile([128, MC], F32, tag="bank", name=f"cps_{which}")
                nc.tensor.matmul(cps, w_s[:, 0, :], yr[:, 0, half], start=True, stop=False)
                nc.tensor.matmul(cps, w_s[:, 1, :], yr[:, 1, half], start=False, stop=True)
                # 2x2 maxpool in one reduce: (128, 4ph, 32pw, 2hh, 2ww) -> XY
                v = cps.rearrange("p (ph hh pw ww) -> p ph pw hh ww", ph=4, hh=2, ww=2)
                nc.vector.tensor_reduce(
                    out=dst[:, mc * 4:(mc + 1) * 4, :], in_=v, axis=AX.XY, op=ALU.max,
                )

        if with_theta:
            theta_phase(b)

        # ---------------- gT tiles with [ones | zeros] pad columns ----------
        # gt: (128 pooled, 8 pchunk, 132) ; [:, :, 0:128]=g^T, col 128=1, rest 0
        gt = gtp.tile([128, 8, GW], BF16, tag="gt")
        nc.sync.dma_start(out=gt[:, :, 128:GW], in_=gpad[:, :, 0:GW - 128])
        g_flat = g_p.rearrange("p a b -> p (a b)")
        for half in range(2):
            gtps = pbank.tile([128, 4, 128], BF16, tag="bank", name="gtps")
            for j in range(4):
                pch = half * 4 + j
                nc.tensor.transpose(
                    gtps[:, j, :], g_flat[:, pch * 128:(pch + 1) * 128],
                    identb_s,
                )
            nc.vector.tensor_copy(out=gt[:, half * 4:(half + 1) * 4, 0:128], in_=gtps)

        phi_tiles[b] = phi_p
        gt_tiles[b] = gt

    # ---------------- attention per item / m-chunk ----------------
    # item b+1's conv phase is injected midway through item b's attention so
    # neither the PE nor the ACT exp chain drains at the item boundary, while
    # attention 0 still starts as soon as item 0's inputs land.
    def attention_phase(b, inject_mc=None, inject_fn=None):
        theta = theta_tiles[b]
        zt = ztp.tile([128, N], BF16, tag="zt", name=f"zt_{b}")
        zt_tiles[b] = zt
        phi_flat = phi_tiles[b].rearrange("p a b -> p (a b)")
        gt = gt_tiles[b]
        for mc in range(NMC):
            if mc == inject_mc and inject_fn is not None:
                inject_fn()
            ms = slice(mc * MC, (mc + 1) * MC)
            # fT tiles: (128 pooled, 512 m) for each of 8 pooled chunks; exp on ACT
            fexp = fxp.tile([128, 8, MC], BF16, tag="fexp")
            for half in range(4):
                fps = psf.tile([128, 2, MC], F32, tag="f")
                for i in range(2):
                    pch = half * 2 + i
                    nc.tensor.matmul(
                        fps[:, i, :],
                        phi_flat[:, pch * 128:(pch + 1) * 128],
                        theta[:, ms],
                        start=True, stop=True,
                    )
                nc.scalar.activation(fexp[:, 2 * half:2 * half + 2, :], fps, ACT.Exp)

            # z matmuls: out (128 m, [z | s | junk]) accumulated over 8 pooled
            # chunks; two m-subtiles per PSUM bank tile
            tp = pbank.tile([128, 4, 128], BF16, tag="bank", name="tp")
            for j2 in range(2):
                zb = pbank.tile([128, 512], F32, tag="bank", name="zb")
                for i in range(2):
                    sub = j2 * 2 + i
                    for pch in range(8):
                        nc.tensor.matmul(
                            zb[:, i * 256:i * 256 + GW],
                            fexp[:, pch, sub * 128:(sub + 1) * 128],
                            gt[:, pch, :],
                            start=(pch == 0), stop=(pch == 7),
                        )
                zb2 = zb.rearrange("p (i c) -> p i c", i=2)
                rc = ptmp.tile([128, 2], F32, tag="rc", bufs=6)
                nc.vector.reciprocal(rc, zb2[:, :, 128])
                zn2 = znp.tile([128, 2, 128], BF16, tag="zn")
                nc.vector.tensor_tensor(
                    zn2, zb2[:, :, 0:128],
                    rc[:, :, None].to_broadcast((128, 2, 128)), ALU.mult,
                )
                for i in range(2):
                    sub = j2 * 2 + i
                    nc.tensor.transpose(tp[:, sub, :], zn2[:, i, :], identb_s)
                    # Gram accumulation for BN variance: ZZ += z_m^T z_m
                    nc.tensor.matmul(
                        zz_ps, zn2[:, i, :], zn2[:, i, :],
                        start=(zz_n[0] == 0), stop=(zz_n[0] == zz_last),
                        skip_group_check=True,
                    )
                    zz_n[0] += 1
            # zt copy doubles as the per-chunk z column-sum for the BN mean
            nc.vector.tensor_scalar(
                zt[:, ms], tp.rearrange("p a b -> p (a b)"), 1.0, 0.0,
                ALU.mult, ALU.add, accum_out=acc_s[:, b * NMC + mc:b * NMC + mc + 1],
            )


    conv_phase(0)
    conv_phase(1)
    attention_phase(0)
    attention_phase(1)

    # ---------------- global BN stats via AllReduce ----------------
    # ls[:, cc] = sum(wz), ls[:, 2+cc] = sum(wz^2), both derived from z
    ls = consts.tile([128, 4], F32)
    sumz = consts.tile([128, 2], F32R)
    with nc.allow_low_precision(reason="f32r is full-width f32 storage"):
        nc.vector.reduce_sum(out=sumz[:, 0:1], in_=acc_s, axis=AX.X)
        nc.vector.reduce_sum(out=sumz[:, 1:2], in_=acc_s[:, 0:1], axis=AX.X)
    # sum(wz)_c = W[c,:] @ sumz ; sum(wz^2)_c = diag(W ZZ W^T)
    zz_s = consts.tile([128, 128], F32R)
    nc.vector.tensor_copy(out=zz_s, in_=zz_ps)
    for cc in range(2):
        s1_ps = pbank.tile([128, 2], F32, tag="bank", name="s1_ps")
        nc.tensor.matmul(s1_ps, wwr_s[:, cc * 128:(cc + 1) * 128], sumz,
                         start=True, stop=True)
        nc.vector.tensor_copy(out=ls[:, cc:cc + 1], in_=s1_ps[:, 0:1])
        u_ps = pbank.tile([128, 128], F32, tag="bank", name="u_ps")
        nc.tensor.matmul(u_ps, wwr_s[:, cc * 128:(cc + 1) * 128], zz_s,
                         start=True, stop=True)
        qjunk = ptmp.tile([128, 128], F32, tag="qjunk", bufs=1)
        nc.vector.scalar_tensor_tensor(
            qjunk, u_ps, 1.0, wraw_s[:, cc, :], ALU.mult, ALU.mult,
            accum_out=ls[:, 2 + cc:3 + cc],
        )

    cc_in = dram.tile([128, 4], F32)
    cc_out = dram.tile([128, 4], F32)
    nc.sync.dma_start(out=cc_in, in_=ls)

    if io.get("single_core_sim"):
        # stand-in for the AllReduce so TimelineSim (single-core) can run
        nc.sync.dma_start(out=cc_out, in_=cc_in)
    else:
        nc.gpsimd.collective_compute(
            "AllReduce", ALU.add,
            replica_groups=[list(range(NCORES))],
            ins=[cc_in.opt()], outs=[cc_out.opt()],
        )
    gs = consts.tile([128, 4], F32)
    nc.sync.dma_start(out=gs, in_=cc_out)

    inv = 1.0 / COUNT
    st4 = consts.tile([128, 4], F32)
    nc.vector.tensor_scalar(st4, gs, inv, None, ALU.mult)
    mean = st4[:, 0:2]
    e2 = st4[:, 2:4]
    msq = consts.tile([128, 2], F32)
    nc.vector.tensor_mul(msq, mean, mean)
    u = consts.tile([128, 2], F32)
    nc.vector.tensor_sub(u, e2, msq)
    nc.vector.tensor_scalar(u, u, EPS, None, ALU.add)
    # rsqrt(u) = exp(-0.5*ln(u)) -- one ACT table switch, costs ~1.3us
    y0 = consts.tile([128, 2], F32)
    nc.scalar.activation(y0, u, ACT.Ln)
    r0 = consts.tile([128, 2], F32)
    nc.scalar.activation(r0, y0, ACT.Exp, scale=-0.5)
    a_s = consts.tile([128, 2], F32)
    nc.vector.tensor_mul(a_s, r0, gamma_s)
    nb = consts.tile([128, 2], F32)
    nc.vector.tensor_mul(nb, mean, a_s)
    nc.vector.tensor_sub(nb, beta_s, nb)

    # ---------------- W conv + normalize + residual + store ----------------
    for b in range(BLOC):
        x_t = x_tiles[b]
        zt = zt_tiles[b]
        for mc in range(NMC):
            ms = slice(mc * MC, (mc + 1) * MC)
            for cc in range(2):
                csl = slice(cc * 128, (cc + 1) * 128)
                wb = pbank.tile([128, MC], F32, tag="bank", name="wb")
                nc.tensor.matmul(
                    wb, ww_s[:, cc * 128:(cc + 1) * 128], zt[:, ms],
                    start=True, stop=True,
                )
                # normalize on ACT: wn = wz*a + (beta - mean*a)
                wn = wnp.tile([128, MC], BF16, tag="wn")
                nc.scalar.activation(
                    wn, wb, ACT.Identity,
                    bias=nb[:, cc:cc + 1], scale=a_s[:, cc:cc + 1],
                )
                ot = outp.tile([128, MC], F32, tag="ot")
                eng = nc.gpsimd if (2 * mc + cc) % 4 == 3 else nc.vector
                eng.tensor_add(ot, wn, x_t[:, cc, ms])
                nc.sync.dma_start(out=out[b, csl, ms], in_=ot)


def make_io(nc):
    return {
        "x": nc.dram_tensor("x", [BLOC, C, N], BF16, kind="ExternalInput").ap(),
        "y": nc.dram_tensor("y", [BLOC, C, N], BF16, kind="ExternalInput").ap(),
        "wpack": nc.dram_tensor("wpack", [128, 1152], BF16, kind="ExternalInput").ap(),
        "vpack": nc.dram_tensor("vpack", [128, 262], F32, kind="ExternalInput").ap(),
        "wwr": nc.dram_tensor("wwr", [128, 256], mybir.dt.float32r, kind="ExternalInput").ap(),
        "gpad": nc.dram_tensor("gpad", [128, 8, 4], BF16, kind="ExternalInput").ap(),
        "out": nc.dram_tensor("out", [BLOC, C, N], F32, kind="ExternalOutput").ap(),
    }


_CACHE = {}


def _get_program():
    if "nc" in _CACHE:
        return _CACHE["nc"], _CACHE["io"]
    nc = bacc.Bacc(
        "TRN2", target_bir_lowering=False, debug=False,
        enable_asserts=False, num_devices=NCORES,
    )
    io = make_io(nc)
    from contextlib import ExitStack
    with tile.TileContext(nc) as tc:
        with ExitStack() as ctx:
            io["ctx"] = ctx
            build_body(tc, io)
    nc.compile()
    _CACHE["nc"] = nc
    _CACHE["io"] = io
    return nc, io


def kernel(x, y, theta_w, theta_b, phi_w, phi_b, g_w, g_b, W_w, W_b,
           bn_gamma, bn_beta, _trace=False, **_unused):
    x = np.asarray(x, dtype=np.float32).reshape(B, C, N).astype(ml_bf16)
    y = np.asarray(y, dtype=np.float32).reshape(B, C, N).astype(ml_bf16)

    def chunked(wT):
        # (C, CI) -> (128, 2, CI): [p, k, ci] = wT[k*128+p, ci]
        return np.asarray(wT, np.float32).reshape(2, 128, CI).transpose(1, 0, 2)

    tw = chunked(np.asarray(theta_w, np.float32).T)
    pw = chunked(np.asarray(phi_w, np.float32).T)
    gw = chunked(np.asarray(g_w, np.float32).T)
    ww = np.asarray(W_w, np.float32).T                             # (CI, C)
    wraw = chunked(np.asarray(W_w, np.float32))                    # c-part layout
    ident = np.eye(128, dtype=np.float32)
    wpack = np.ascontiguousarray(np.concatenate([
        tw.reshape(128, 256), pw.reshape(128, 256), gw.reshape(128, 256),
        ww, ident], axis=1).astype(ml_bf16))
    tb = np.asarray(theta_b, np.float32).reshape(CI, 1)
    gamma = np.asarray(bn_gamma, np.float32).reshape(2, 128).T
    beta = np.asarray(bn_beta, np.float32).reshape(2, 128).T
    vpack = np.ascontiguousarray(np.concatenate(
        [tb, gamma, beta, wraw.reshape(128, 256),
         np.full((128, 1), EPS, np.float32)], axis=1))
    wwr = np.ascontiguousarray(ww)
    gpad = np.zeros((128, 8, 4), ml_bf16)
    gpad[:, :, 0] = 1.0
    # phi_b, g_b, W_b intentionally unused: softmax-invariant / cancelled by BN.

    nc, _ = _get_program()
    in_maps = []
    for k in range(NCORES):
        in_maps.append({
            "x": np.ascontiguousarray(x[k * BLOC:(k + 1) * BLOC]),
            "y": np.ascontiguousarray(y[k * BLOC:(k + 1) * BLOC]),
            "wpack": wpack, "vpack": vpack, "wwr": wwr, "gpad": gpad,
        })
    res = run_bass_kernel_spmd(nc, in_maps, core_ids=list(range(NCORES)), trace=_trace)
    out = np.concatenate([r_["out"] for r_ in res.results], axis=0)
    if _trace:
        _CACHE["last_results"] = res
    return out.reshape(B, C, 64, 64)


# revision 48
# speedup vs baseline: 1.0632x; 1.0065x over previous
# Trainium2 Bass kernel for the non-local attention block (nn_DRAL_88476326297980).
#
# Reference computation (per batch b):
#   theta = theta_w @ x_b + theta_b            (CI=128, N=4096)
#   phi   = maxpool2x2(phi_w @ y_b + phi_b)    (CI=128, P=1024)
#   g     = maxpool2x2(g_w  @ y_b + g_b)       (CI=128, P=1024)
#   f     = theta^T @ phi                      (N, P)
#   fdiv  = softmax(f, axis=P)
#   z     = fdiv @ g^T                         (N, CI)
#   wz    = W_w @ z^T + W_b                    (C=256, N)
#   out   = BN(wz over all b,n) + x            (training-mode batch stats)
#
# Sharding: data-parallel over batch, 2 batches per core, 8 cores.
# BN batch statistics are combined with a tiny (128x4) AllReduce.
#
# Math simplifications used (exact, not approximations):
#  - phi_b adds a per-row constant to f -> softmax-invariant -> dropped.
#  - g_b adds a per-CI constant to z (softmax weights sum to 1) -> shifts wz
#    per-channel -> cancelled by the BN mean subtraction -> dropped.
#  - W_b shifts wz per-channel -> cancelled by BN mean subtraction -> dropped.
#  - BN statistics are computed from z (before the W conv):
#      sum(wz)_c  = W @ sum_m(z_m)     (sum_m z from accum on the zt copies)
#      sum(wz^2)_c = diag(W ZZ W^T)    (ZZ = z Gram matrix, accumulated in PSUM)
#    so the AllReduce fires right after the last attention tile and the
#    W conv + normalize + residual + store run as one fused pipeline.
#
# Layout choices:
#  - everything on the PE runs in bf16 (full rate at any tile size; f32r pays
#    4x below 256 output cols, which hit the old Gram/transpose path).
#  - f is computed TRANSPOSED (fT: pooled dim on partitions, n on free) so both
#    attention matmuls contract over the partition dim with no transposes of f.
#  - softmax denominators come from an extra all-ones column appended to the
#    g^T tiles: the z-matmul then yields [z_unnorm | s | junk] in one PSUM
#    accumulation group.
#  - x and y are loaded as bf16 (host-converted), halving input HBM traffic.

import numpy as np
from ml_dtypes import bfloat16 as ml_bf16

import concourse.bass as bass
import concourse.mybir as mybir
import concourse.tile as tile
from concourse import bacc
from concourse.bass_utils import run_bass_kernel_spmd

F32 = mybir.dt.float32
F32R = mybir.dt.float32r
BF16 = mybir.dt.bfloat16
ALU = mybir.AluOpType
ACT = mybir.ActivationFunctionType
AX = mybir.AxisListType

NCORES = 8
B = 16
BLOC = B // NCORES          # 2 batches per core
C = 256                     # in channels
CI = 128                    # inter channels
N = 4096                    # h*w
MC = 512                    # m-chunk (columns per matmul)
NMC = N // MC               # 8
GW = 130                    # z-matmul output width: 128 z cols + s + 1 junk
EPS = 1e-5
COUNT = B * N               # BN sample count per channel


def build_body(tc, io):
    nc = tc.nc
    x, y, wpack, vpack, gpad, out = (
        io["x"], io["y"], io["wpack"], io["vpack"], io["gpad"], io["out"],
    )

    ctx = io["ctx"]
    consts = ctx.enter_context(tc.tile_pool(name="consts", bufs=1))
    xfp = ctx.enter_context(tc.tile_pool(name="xfp", bufs=2))
    yin = ctx.enter_context(tc.tile_pool(name="yin", bufs=8))
    thp = ctx.enter_context(tc.tile_pool(name="thp", bufs=2))
    poolp = ctx.enter_context(tc.tile_pool(name="poolp", bufs=2))
    ptmp = ctx.enter_context(tc.tile_pool(name="ptmp", bufs=1))
    gtp = ctx.enter_context(tc.tile_pool(name="gtp", bufs=2))
    fxp = ctx.enter_context(tc.tile_pool(name="fxp", bufs=3))
    znp = ctx.enter_context(tc.tile_pool(name="znp", bufs=8))
    ztp = ctx.enter_context(tc.tile_pool(name="ztp", bufs=2))
    wzp = ctx.enter_context(tc.tile_pool(name="wzp", bufs=16))
    wnp = ctx.enter_context(tc.tile_pool(name="wnp", bufs=6))
    outp = ctx.enter_context(tc.tile_pool(name="outp", bufs=8))
    psf = ctx.enter_context(tc.tile_pool(name="psf", bufs=2, space="PSUM"))
    pbank = ctx.enter_context(tc.tile_pool(name="pbank", bufs=3, space="PSUM"))
    pzz = ctx.enter_context(tc.tile_pool(name="pzz", bufs=1, space="PSUM"))
    dram = ctx.enter_context(tc.tile_pool(name="dram", bufs=1, space="DRAM"))

    # ---- constants / weights: two packed DMAs to keep sync fan-in tiny ----
    # wpack (128, 1152) bf16:
    #   [twT(2x128) pwT(2x128) gwT(2x128) wwT(256) identb(128)]
    wp_s = consts.tile([128, 1152], BF16)
    nc.sync.dma_start(out=wp_s[:, 0:768], in_=wpack[:, 0:768])
    nc.sync.dma_start(out=wp_s[:, 768:1152], in_=wpack[:, 768:1152])
    tw_s = wp_s[:, 0:256].rearrange("p (k c) -> p k c", k=2)
    pw_s = wp_s[:, 256:512].rearrange("p (k c) -> p k c", k=2)
    gw_s = wp_s[:, 512:768].rearrange("p (k c) -> p k c", k=2)
    ww_s = wp_s[:, 768:1024]
    identb_s = wp_s[:, 1024:1152]
    # vpack (128, 261) f32: [tb, gamma(2), beta(2), wraw(2x128), wwT(256)]
    vp_s = consts.tile([128, 262], F32)
    nc.sync.dma_start(out=vp_s, in_=vpack)
    tb_s = vp_s[:, 0:1]
    gamma_s = vp_s[:, 1:3]
    beta_s = vp_s[:, 3:5]
    wraw_s = vp_s[:, 5:261].rearrange("p (k c) -> p k c", k=2)
    eps_s = vp_s[:, 261:262]
    wwr_s = consts.tile([128, 256], F32R)
    nc.sync.dma_start(out=wwr_s, in_=io["wwr"])

    acc_s = consts.tile([128, BLOC * NMC], F32)       # per (b, mc) z sums
    zz_ps = pzz.tile([128, 128], F32)                 # z Gram matrix accumulator

    zz_n = [0]
    zz_last = BLOC * NMC * 4 - 1                      # 64 accumulated Gram matmuls

    # ---------------- input DMAs for both items, issued upfront -------------
    # y[b] first (the attention loop needs the full phi before it can start),
    # then x[b]; item 1's loads stream in behind item 0's while item 0
    # computes, so the item transition pays no DMA latency.
    x_tiles = {}
    y_tiles_all = {}
    for b in range(BLOC):
        ys = y[b].rearrange("(k p) m -> p k m", p=128)
        y_tiles_all[b] = []
        for q in range(4):
            yr = yin.tile([128, 2, 2 * MC], BF16, tag="yin", name=f"y_{b}_{q}")
            if b == 0 and q == 0:
                nc.sync.dma_start(out=yr[:, :, 0:MC], in_=ys[:, :, 0:MC])
                nc.sync.dma_start(out=yr[:, :, MC:2 * MC], in_=ys[:, :, MC:2 * MC])
            else:
                nc.sync.dma_start(out=yr, in_=ys[:, :, q * 2 * MC:(q + 1) * 2 * MC])
            y_tiles_all[b].append(yr)

        x_t = xfp.tile([128, 2, N], BF16, tag="xf", name=f"x_{b}")
        x_tiles[b] = x_t
        xs = x[b].rearrange("(k p) m -> p k m", p=128)
        for q in range(4):
            qs = slice(q * (N // 4), (q + 1) * (N // 4))
            nc.sync.dma_start(out=x_t[:, :, qs], in_=xs[:, :, qs])

    zt_tiles = {}
    phi_tiles = {}
    gt_tiles = {}
    theta_tiles = {}

    def theta_phase(b):
        x_t = x_tiles[b]
        # ---------------- theta conv: (128ci, 4096) ----------------
        theta = thp.tile([128, N], F32R, tag="theta")
        for mc in range(NMC):
            ms = slice(mc * MC, (mc + 1) * MC)
            tps = pbank.tile([128, MC], F32, tag="bank", name="tps")
            nc.tensor.matmul(tps, tw_s[:, 0, :], x_t[:, 0, ms], start=True, stop=False)
            nc.tensor.matmul(tps, tw_s[:, 1, :], x_t[:, 1, ms], start=False, stop=True)
            nc.scalar.activation(theta[:, ms], tps, ACT.Identity, bias=tb_s, scale=1.0)
        theta_tiles[b] = theta

    def conv_phase(b, with_theta=True):
        x_t = x_tiles[b]
        y_tiles = y_tiles_all[b]

        # ---------------- phi/g convs + 2x2 maxpool ----------------
        # pooled tensors: (128ci, 32ph, 32pw)
        phi_p = poolp.tile([128, 32, 32], F32R, tag="phi_p")
        g_p = poolp.tile([128, 32, 32], BF16, tag="g_p")
        for mc in range(NMC):
            yr = y_tiles[mc // 2]
            half = slice((mc % 2) * MC, (mc % 2 + 1) * MC)
            for which, w_s, dst in (("phi", pw_s, phi_p), ("g", gw_s, g_p)):
                cps = pbank.tile([128, MC], F32, tag="bank", name=f"cps_{which}")
                nc.tensor.matmul(cps, w_s[:, 0, :], yr[:, 0, half], start=True, stop=False)
                nc.tensor.matmul(cps, w_s[:, 1, :], yr[:, 1, half], start=False, stop=True)
                # 2x2 maxpool in one reduce: (128, 4ph, 32pw, 2hh, 2ww) -> XY
                v = cps.rearrange("p (ph hh pw ww) -> p ph pw hh ww", ph=4, hh=2, ww=2)
                nc.vector.tensor_reduce(
                    out=dst[:, mc * 4:(mc + 1) * 4, :], in_=v, axis=AX.XY, op=ALU.max,
                )

        if with_theta:
            theta_phase(b)

        # ---------------- gT tiles with [ones | zeros] pad columns ----------
        # gt: (128 pooled, 8 pchunk, 132) ; [:, :, 0:128]=g^T, col 128=1, rest 0
        gt = gtp.tile([128, 8, GW], BF16, tag="gt")
        nc.sync.dma_start(out=gt[:, :, 128:GW], in_=gpad[:, :, 0:GW - 128])
        g_flat = g_p.rearrange("p a b -> p (a b)")
        for half in range(2):
            gtps = pbank.tile([128, 4, 128], BF16, tag="bank", name="gtps")
            for j in range(4):
                pch = half * 4 + j
                nc.tensor.transpose(
                    gtps[:, j, :], g_flat[:, pch * 128:(pch + 1) * 128],
                    identb_s,
                )
            nc.vector.tensor_copy(out=gt[:, half * 4:(half + 1) * 4, 0:128], in_=gtps)

        phi_tiles[b] = phi_p
        gt_tiles[b] = gt

    # ---------------- attention per item / m-chunk ----------------
    # item b+1's conv phase is injected midway through item b's attention so
    # neither the PE nor the ACT exp chain drains at the item boundary, while
    # attention 0 still starts as soon as item 0's inputs land.
    def attention_phase(b, inject_mc=None, inject_fn=None):
        theta = theta_tiles[b]
        zt = ztp.tile([128, N], BF16, tag="zt", name=f"zt_{b}")
        zt_tiles[b] = zt
        phi_flat = phi_tiles[b].rearrange("p a b -> p (a b)")
        gt = gt_tiles[b]
        for mc in range(NMC):
            if mc == inject_mc and inject_fn is not None:
                inject_fn()
            ms = slice(mc * MC, (mc + 1) * MC)
            # fT tiles: (128 pooled, 512 m) for each of 8 pooled chunks; exp on ACT
            fexp = fxp.tile([128, 8, MC], BF16, tag="fexp")
            for half in range(4):
                fps = psf.tile([128, 2, MC], F32, tag="f")
                for i in range(2):
                    pch = half * 2 + i
                    nc.tensor.matmul(
                        fps[:, i, :],
                        phi_flat[:, pch * 128:(pch + 1) * 128],
                        theta[:, ms],
                        start=True, stop=True,
                    )
                nc.scalar.activation(fexp[:, 2 * half:2 * half + 2, :], fps, ACT.Exp)

            # z matmuls: out (128 m, [z | s | junk]) accumulated over 8 pooled
            # chunks; two m-subtiles per PSUM bank tile
            tp = pbank.tile([128, 4, 128], BF16, tag="bank", name="tp")
            for j2 in range(2):
                zb = pbank.tile([128, 512], F32, tag="bank", name="zb")
                for i in range(2):
                    sub = j2 * 2 + i
                    for pch in range(8):
                        nc.tensor.matmul(
                            zb[:, i * 256:i * 256 + GW],
                            fexp[:, pch, sub * 128:(sub + 1) * 128],
                            gt[:, pch, :],
                            start=(pch == 0), stop=(pch == 7),
                        )
                zb2 = zb.rearrange("p (i c) -> p i c", i=2)
                rc = ptmp.tile([128, 2], F32, tag="rc", bufs=6)
                nc.vector.reciprocal(rc, zb2[:, :, 128])
                zn2 = znp.tile([128, 2, 128], BF16, tag="zn")
                nc.vector.tensor_tensor(
                    zn2, zb2[:, :, 0:128],
                    rc[:, :, None].to_broadcast((128, 2, 128)), ALU.mult,
                )
                for i in range(2):
                    sub = j2 * 2 + i
                    nc.tensor.transpose(tp[:, sub, :], zn2[:, i, :], identb_s)
                    # Gram accumulation for BN variance: ZZ += z_m^T z_m
                    nc.tensor.matmul(
                        zz_ps, zn2[:, i, :], zn2[:, i, :],
                        start=(zz_n[0] == 0), stop=(zz_n[0] == zz_last),
                        skip_group_check=True,
                    )
                    zz_n[0] += 1
            # zt copy doubles as the per-chunk z column-sum for the BN mean
            nc.vector.tensor_scalar(
                zt[:, ms], tp.rearrange("p a b -> p (a b)"), 1.0, 0.0,
                ALU.mult, ALU.add, accum_out=acc_s[:, b * NMC + mc:b * NMC + mc + 1],
            )


    conv_phase(0)
    conv_phase(1)
    attention_phase(0)
    attention_phase(1)


    # ---------------- global BN stats via AllReduce ----------------
    # ls[:, cc] = sum(wz), ls[:, 2+cc] = sum(wz^2), both derived from z
    ls = consts.tile([128, 4], F32)
    sumz = consts.tile([128, 2], F32R)
    with nc.allow_low_precision(reason="f32r is full-width f32 storage"):
        nc.vector.reduce_sum(out=sumz[:, 0:1], in_=acc_s, axis=AX.X)
        nc.vector.reduce_sum(out=sumz[:, 1:2], in_=acc_s[:, 0:1], axis=AX.X)
    # sum(wz)_c = W[c,:] @ sumz ; sum(wz^2)_c = diag(W ZZ W^T)
    zz_s = consts.tile([128, 128], F32R)
    nc.vector.tensor_copy(out=zz_s, in_=zz_ps)
    for cc in range(2):
        s1_ps = pbank.tile([128, 2], F32, tag="bank", name="s1_ps")
        nc.tensor.matmul(s1_ps, wwr_s[:, cc * 128:(cc + 1) * 128], sumz,
                         start=True, stop=True)
        nc.vector.tensor_copy(out=ls[:, cc:cc + 1], in_=s1_ps[:, 0:1])
        u_ps = pbank.tile([128, 128], F32, tag="bank", name="u_ps")
        nc.tensor.matmul(u_ps, wwr_s[:, cc * 128:(cc + 1) * 128], zz_s,
                         start=True, stop=True)
        qjunk = ptmp.tile([128, 128], F32, tag="qjunk", bufs=1)
        nc.vector.scalar_tensor_tensor(
            qjunk, u_ps, 1.0, wraw_s[:, cc, :], ALU.mult, ALU.mult,
            accum_out=ls[:, 2 + cc:3 + cc],
        )

    cc_in = dram.tile([128, 4], F32)
    cc_out = dram.tile([128, 4], F32)
    nc.sync.dma_start(out=cc_in, in_=ls)

    if io.get("single_core_sim"):
        # stand-in for the AllReduce so TimelineSim (single-core) can run
        nc.sync.dma_start(out=cc_out, in_=cc_in)
    else:
        nc.gpsimd.collective_compute(
            "AllReduce", ALU.add,
            replica_groups=[list(range(NCORES))],
            ins=[cc_in.opt()], outs=[cc_out.opt()],
        )
    gs = consts.tile([128, 4], F32)
    nc.sync.dma_start(out=gs, in_=cc_out)

    inv = 1.0 / COUNT
    mean = consts.tile([128, 2], F32)
    nc.vector.tensor_scalar(mean, gs[:, 0:2], inv, None, ALU.mult)
    e2p = consts.tile([128, 2], F32)
    nc.vector.tensor_scalar(e2p, gs[:, 2:4], inv, EPS, ALU.mult, ALU.add)
    msq = consts.tile([128, 2], F32)
    nc.vector.tensor_mul(msq, mean, mean)
    u = consts.tile([128, 2], F32)
    # u = e2 + eps - mean^2
    nc.vector.scalar_tensor_tensor(u, msq, -1.0, e2p, ALU.mult, ALU.add)
    # rsqrt(u) = exp(-0.5*ln(u)) -- one ACT table switch, costs ~1.3us
    y0 = consts.tile([128, 2], F32)
    nc.scalar.activation(y0, u, ACT.Ln)
    r0 = consts.tile([128, 2], F32)
    nc.scalar.activation(r0, y0, ACT.Exp, scale=-0.5)
    a_s = consts.tile([128, 2], F32)
    nc.vector.tensor_mul(a_s, r0, gamma_s)
    nb = consts.tile([128, 2], F32)
    nc.vector.tensor_mul(nb, mean, a_s)
    nc.vector.tensor_sub(nb, beta_s, nb)

    # ---------------- W conv + normalize + residual + store ----------------
    for b in range(BLOC):
        x_t = x_tiles[b]
        zt = zt_tiles[b]
        for mc in range(NMC):
            ms = slice(mc * MC, (mc + 1) * MC)
            for cc in range(2):
                csl = slice(cc * 128, (cc + 1) * 128)
                wb = pbank.tile([128, MC], F32, tag="bank", name="wb")
                nc.tensor.matmul(
                    wb, ww_s[:, cc * 128:(cc + 1) * 128], zt[:, ms],
                    start=True, stop=True,
                )
                # normalize on ACT: wn = wz*a + (beta - mean*a)
                wn = wnp.tile([128, MC], BF16, tag="wn")
                nc.scalar.activation(
                    wn, wb, ACT.Identity,
                    bias=nb[:, cc:cc + 1], scale=a_s[:, cc:cc + 1],
                )
                ot = outp.tile([128, MC], F32, tag="ot")
                eng = nc.gpsimd if (2 * mc + cc) % 4 == 3 else nc.vector
                eng.tensor_add(ot, wn, x_t[:, cc, ms])
                nc.sync.dma_start(out=out[b, csl, ms], in_=ot)


def make_io(nc):
    return {
        "x": nc.dram_tensor("x", [BLOC, C, N], BF16, kind="ExternalInput").ap(),
        "y": nc.dram_tensor("y", [BLOC, C, N], BF16, kind="ExternalInput").ap(),
        "wpack": nc.dram_tensor("wpack", [128, 1152], BF16, kind="ExternalInput").ap(),
        "vpack": nc.dram_tensor("vpack", [128, 262], F32, kind="ExternalInput").ap(),
        "wwr": nc.dram_tensor("wwr", [128, 256], mybir.dt.float32r, kind="ExternalInput").ap(),
        "gpad": nc.dram_tensor("gpad", [128, 8, 4], BF16, kind="ExternalInput").ap(),
        "out": nc.dram_tensor("out", [BLOC, C, N], F32, kind="ExternalOutput").ap(),
    }


_CACHE = {}


def _get_program():
    if "nc" in _CACHE:
        return _CACHE["nc"], _CACHE["io"]
    nc = bacc.Bacc(
        "TRN2", target_bir_lowering=False, debug=False,
        enable_asserts=False, num_devices=NCORES,
    )
    io = make_io(nc)
    from contextlib import ExitStack
    with tile.TileContext(nc) as tc:
        with ExitStack() as ctx:
            io["ctx"] = ctx
            build_body(tc, io)
    nc.compile()
    _CACHE["nc"] = nc
    _CACHE["io"] = io
    return nc, io


def kernel(x, y, theta_w, theta_b, phi_w, phi_b, g_w, g_b, W_w, W_b,
           bn_gamma, bn_beta, _trace=False, **_unused):
    x = np.asarray(x, dtype=np.float32).reshape(B, C, N).astype(ml_bf16)
    y = np.asarray(y, dtype=np.float32).reshape(B, C, N).astype(ml_bf16)

    def chunked(wT):
        # (C, CI) -> (128, 2, CI): [p, k, ci] = wT[k*128+p, ci]
        return np.asarray(wT, np.float32).reshape(2, 128, CI).transpose(1, 0, 2)

    tw = chunked(np.asarray(theta_w, np.float32).T)
    pw = chunked(np.asarray(phi_w, np.float32).T)
    gw = chunked(np.asarray(g_w, np.float32).T)
    ww = np.asarray(W_w, np.float32).T                             # (CI, C)
    wraw = chunked(np.asarray(W_w, np.float32))                    # c-part layout
    ident = np.eye(128, dtype=np.float32)
    wpack = np.ascontiguousarray(np.concatenate([
        tw.reshape(128, 256), pw.reshape(128, 256), gw.reshape(128, 256),
        ww, ident], axis=1).astype(ml_bf16))
    tb = np.asarray(theta_b, np.float32).reshape(CI, 1)
    gamma = np.asarray(bn_gamma, np.float32).reshape(2, 128).T
    beta = np.asarray(bn_beta, np.float32).reshape(2, 128).T
    vpack = np.ascontiguousarray(np.concatenate(
        [tb, gamma, beta, wraw.reshape(128, 256),
         np.full((128, 1), EPS, np.float32)], axis=1))
    wwr = np.ascontiguousarray(ww)
    gpad = np.zeros((128, 8, 4), ml_bf16)
    gpad[:, :, 0] = 1.0
    # phi_b, g_b, W_b intentionally unused: softmax-invariant / cancelled by BN.

    nc, _ = _get_program()
    in_maps = []
    for k in range(NCORES):
        in_maps.append({
            "x": np.ascontiguousarray(x[k * BLOC:(k + 1) * BLOC]),
            "y": np.ascontiguousarray(y[k * BLOC:(k + 1) * BLOC]),
            "wpack": wpack, "vpack": vpack, "wwr": wwr, "gpad": gpad,
        })
    res = run_bass_kernel_spmd(nc, in_maps, core_ids=list(range(NCORES)), trace=_trace)
    out = np.concatenate([r_["out"] for r_ in res.results], axis=0)
    if _trace:
        _CACHE["last_results"] = res
    return out.reshape(B, C, 64, 64)
